# revision 14
# baseline (speedup 1.0000x reference)
"""Trainium2 Bass kernel for nn_ListREPLValueHead (tree NN + RNN value head).

Strategy
--------
Data-parallel over the sketch batch B=512 across 8 NeuronCores (64 sketches,
N = 64*8 = 512 example-columns per core).  All weights replicated.

All activations live in *transposed* layout [H on partitions, batch on free]:
every GEMM is then `out' = W.T-matmul(act')` with the native weight matrix as
the stationary lhsT operand and activations as the moving rhs — no transposes
anywhere on device (x is pre-transposed on host; the embedding gather is done
as a one-hot matmul so the gather result is produced directly transposed).

The whole datapath is bf16 (PSUM accumulation fp32): same PE throughput as
fp32r (1 cycle/row) but half the DMA bytes / SBUF footprint, 2x DVE, and much
better behaved on HW inside PSUM accumulation groups.  End-to-end
quantization error ~9e-3 vs the 2e-2 gate.

DMA priority order matters: one serial DMA stream, so the prologue-critical
tensors (iota, ids, emb, w_hole, xT) are issued first and emb is split into 4
chunks so the one-hot GEMM starts after chunk 0 (~3us).  Bulk weights follow,
tail weights (w_cmp/w_o1) prefetch during the main loop.

Pipeline per core:
  1. one-hot(leaf_ids) [V, B*L]  (DVE is_equal vs iota)
  2. embG' = emb.T @ onehot   [H, B*L]   (PE, ACT evacuates)
  3. xw'  = W_hole.T @ x' + b_hole  [H, N]  (PE + ACT)
  4. t = 0..15:   leaf_t' = relu(xw' + embG'[:, :, t] bcast-e) (DVE add,
                         Pool relu)
                  h_t' = tanh(W_ih.T leaf_t' + W_hh.T h_{t-1}' + b)
                         (PE two-pass: all W_ih k-tiles for the 4 m-tiles
                         first, then W_hh — hides the tanh ACT latency of
                         step t under the W_ih matmuls of step t+1)
                  after odd t: fold tree nodes streaming (PE + ACT relu),
                         O(depth) memory
  5. cmp' = relu(W_cmp.T [root'; ctx'] + b_cmp)   (concat via K-slicing)
  6. compared' = max over E  -> [2H, 64]          (DVE strided reduce)
  7. val = softplus(W_o2.T relu(W_o1.T compared' + b_o1) + b_o2)
     as Ln(1+Exp(.)) on ACT.

build_nc(reps>1) repeats the body with weights hoisted out of the loop —
bench-only mode measuring the steady-state compute time via dispatch slope.
"""

import numpy as np
import ml_dtypes

import concourse.bass as bass
import concourse.mybir as mybir
import concourse.tile as tile
from concourse import bacc

F32 = mybir.dt.float32
BF16 = mybir.dt.bfloat16
AF = mybir.ActivationFunctionType
ALU = mybir.AluOpType

NCORES = 8
B, E, L, H, V, D = 512, 8, 16, 512, 512, 4
P = 128
HT = H // P          # 4 h-tiles
VT = V // P          # 4 v-tiles
NB_FULL = B // NCORES  # 64 sketches per core

_WTILE = lambda ap_, kt: ap_.rearrange("(kt p) m -> p kt m", p=P)  # noqa: E731


def _mm(nc, psum, lhsT, rhs, start, stop):
    nc.tensor.matmul(out=psum, lhsT=lhsT, rhs=rhs, start=start, stop=stop)


def build_nc(NB=NB_FULL, reps=1, dbg=False, wdt="bf16", zb=False):  # zb kept for compat, unused
    """Emit the per-core program. NB = sketches per core (64 full size).
    reps>1: benchmark mode — repeat the body, weights loaded once."""
    N = NB * E
    NL = NB * L

    WD = BF16 if wdt == "bf16" else mybir.dt.float32r
    nc = bacc.Bacc("TRN2", target_bir_lowering=False, debug=False)

    dram = {}
    dram["xT"] = nc.dram_tensor("xT", [H, N], WD, kind="ExternalInput").ap()
    dram["idsf"] = nc.dram_tensor("idsf", [NL], F32, kind="ExternalInput").ap()
    dram["iota"] = nc.dram_tensor("iota", [P, VT], F32, kind="ExternalInput").ap()
    dram["emb"] = nc.dram_tensor("emb", [V, H], WD, kind="ExternalInput").ap()
    dram["w_hole"] = nc.dram_tensor("w_hole", [H, H], WD, kind="ExternalInput").ap()
    dram["w_fn"] = nc.dram_tensor("w_fn", [D, 2 * H, H], WD, kind="ExternalInput").ap()
    dram["w_ih"] = nc.dram_tensor("w_ih", [H, H], WD, kind="ExternalInput").ap()
    dram["w_hh"] = nc.dram_tensor("w_hh", [H, H], WD, kind="ExternalInput").ap()
    dram["w_cmp"] = nc.dram_tensor("w_cmp", [2 * H, H], WD, kind="ExternalInput").ap()
    dram["w_o1"] = nc.dram_tensor("w_o1", [2 * H, H], WD, kind="ExternalInput").ap()
    dram["w_o2"] = nc.dram_tensor("w_o2", [H, 1], WD, kind="ExternalInput").ap()
    for bn, shp in [("b_hole", [H]), ("b_fn", [D, H]), ("b_rnn", [H]),
                    ("b_cmp", [H]), ("b_o1", [H]), ("b_o2", [1])]:
        dram[bn] = nc.dram_tensor(bn, shp, F32, kind="ExternalInput").ap()
    val = nc.dram_tensor("val", [NB], F32, kind="ExternalOutput").ap()
    dbg_outs = {}
    if dbg:
        for nm, shape in [
            ("d_xw", [P, HT, N]), ("d_embG", [P, HT, NL]),
            ("d_leaf3", [P, HT, N]), ("d_h0", [P, HT, N]),
            ("d_ctx", [P, HT, N]), ("d_root", [P, HT, N]),
            ("d_cmp", [P, HT, N]), ("d_cpd", [P, 2 * HT, NB]),
            ("d_o1", [P, HT, NB]),
        ]:
            dbg_outs[nm] = nc.dram_tensor(nm, shape, WD, kind="ExternalOutput").ap()

    with tile.TileContext(nc) as tc:
        with (
            tc.tile_pool(name="persist", bufs=1) as pp,
            tc.tile_pool(name="psum", bufs=8, space="PSUM") as ps,
        ):
            if reps == 1:
                _emit(nc, tc, NB, WD, dbg_outs, pp, ps, dram, val, W=None, zb=zb)
            else:
                W = _load_weights(nc, pp, WD, dram, hoist_inputs=True)
                for _ in range(reps):
                    _emit(nc, tc, NB, WD, {}, pp, ps, dram, val, W=W, zb=zb)

    nc.compile()
    return nc


def _load_bias(nc, pool, ap_, tag):
    t = pool.tile([P, HT], F32, tag=tag)
    nc.sync.dma_start(out=t, in_=ap_.rearrange("(t p) -> p t", p=P))
    return t


def _load_weights(nc, pp, WD, dram, hoist_inputs=False):
    """Load everything weight-like into the persistent pool (bench mode:
    emb/w_hole also persist so the rep body only re-loads ids/xT)."""
    W = {}
    W["iota"] = pp.tile([P, VT], F32, name="iota", tag="iota")
    nc.sync.dma_start(out=W["iota"], in_=dram["iota"])
    for bn in ("b_hole", "b_rnn", "b_cmp", "b_o1"):
        W[bn] = _load_bias(nc, pp, dram[bn], bn)
    W["b_fn"] = pp.tile([P, D, HT], F32, name="b_fn", tag="b_fn")
    nc.sync.dma_start(out=W["b_fn"],
                      in_=dram["b_fn"].rearrange("d (t p) -> p d t", p=P))
    W["b_o2"] = pp.tile([1, 1], F32, name="b_o2", tag="b_o2")
    nc.sync.dma_start(out=W["b_o2"], in_=dram["b_o2"][None, :])
    if hoist_inputs:
        W["emb"] = pp.tile([P, VT, H], WD, name="emb", tag="emb")
        for vt in range(VT):
            nc.sync.dma_start(out=W["emb"][:, vt, :],
                              in_=dram["emb"][vt * P:(vt + 1) * P, :])
        W["w_hole"] = pp.tile([P, HT, H], WD, name="w_hole", tag="w_hole")
        nc.sync.dma_start(out=W["w_hole"], in_=_WTILE(dram["w_hole"], HT))
    W["w_ih"] = pp.tile([P, HT, H], WD, name="w_ih", tag="w_ih")
    nc.sync.dma_start(out=W["w_ih"], in_=_WTILE(dram["w_ih"], HT))
    W["w_hh"] = pp.tile([P, HT, H], WD, name="w_hh", tag="w_hh")
    nc.sync.dma_start(out=W["w_hh"], in_=_WTILE(dram["w_hh"], HT))
    W["w_fn"] = [pp.tile([P, 2 * HT, H], WD, name=f"w_fn{d}", tag=f"w_fn{d}")
                 for d in range(D)]
    for d in range(D):
        nc.sync.dma_start(out=W["w_fn"][d], in_=_WTILE(dram["w_fn"][d], 2 * HT))
    W["w_cmp"] = pp.tile([P, 2 * HT, H], WD, name="w_cmp", tag="w_cmp")
    nc.sync.dma_start(out=W["w_cmp"], in_=_WTILE(dram["w_cmp"], 2 * HT))
    W["w_o1"] = pp.tile([P, 2 * HT, H], WD, name="w_o1", tag="w_o1")
    nc.sync.dma_start(out=W["w_o1"], in_=_WTILE(dram["w_o1"], 2 * HT))
    W["w_o2"] = pp.tile([P, HT, 1], WD, name="w_o2", tag="w_o2")
    nc.sync.dma_start(out=W["w_o2"],
                      in_=dram["w_o2"].rearrange("(t p) o -> p t o", p=P))
    return W


def _emit(nc, tc, NB, WD, dbg_outs, pp, ps, dram, val, W=None, zb=False):
    N = NB * E
    NL = NB * L

    def ddump(nm, t):
        if nm in dbg_outs:
            nc.sync.dma_start(out=dbg_outs[nm], in_=t)

    with (
        tc.tile_pool(name="pro", bufs=1) as pro,
        tc.tile_pool(name="ohp", bufs=2) as ohp,
    ):
        # ---------- per-call inputs + (reps==1) priority-ordered weights ----
        if W is None:
            # graded path: interleave weight DMAs in critical-path order
            hoisted = False
            W = {}
            W["iota"] = pp.tile([P, VT], F32, name="iota", tag="iota")
            nc.sync.dma_start(out=W["iota"], in_=dram["iota"])
            ids_sb = pro.tile([P, NL], F32, name="ids", tag="ids")
            nc.sync.dma_start(out=ids_sb, in_=dram["idsf"].partition_broadcast(P))
            emb_sb = pro.tile([P, VT, H], WD, name="emb", tag="emb")
            for vt in range(VT):
                nc.sync.dma_start(out=emb_sb[:, vt, :],
                                  in_=dram["emb"][vt * P:(vt + 1) * P, :])
            for bn in ("b_hole", "b_rnn", "b_cmp", "b_o1"):
                W[bn] = _load_bias(nc, pp, dram[bn], bn)
            W["b_fn"] = pp.tile([P, D, HT], F32, name="b_fn", tag="b_fn")
            nc.sync.dma_start(out=W["b_fn"],
                              in_=dram["b_fn"].rearrange("d (t p) -> p d t", p=P))
            W["b_o2"] = pp.tile([1, 1], F32, name="b_o2", tag="b_o2")
            nc.sync.dma_start(out=W["b_o2"], in_=dram["b_o2"][None, :])
            w_hole_sb = pro.tile([P, HT, H], WD, name="w_hole", tag="w_hole")
            nc.sync.dma_start(out=w_hole_sb, in_=_WTILE(dram["w_hole"], HT))
            xT_sb = pro.tile([P, HT, N], WD, name="xT", tag="xT")
            nc.sync.dma_start(out=xT_sb, in_=_WTILE(dram["xT"], HT))
            W["w_ih"] = pp.tile([P, HT, H], WD, name="w_ih", tag="w_ih")
            nc.sync.dma_start(out=W["w_ih"], in_=_WTILE(dram["w_ih"], HT))
            W["w_hh"] = pp.tile([P, HT, H], WD, name="w_hh", tag="w_hh")
            nc.sync.dma_start(out=W["w_hh"], in_=_WTILE(dram["w_hh"], HT))
            W["w_fn"] = [pp.tile([P, 2 * HT, H], WD, name=f"w_fn{d}",
                                 tag=f"w_fn{d}") for d in range(D)]
            for d in range(D):
                nc.sync.dma_start(out=W["w_fn"][d],
                                  in_=_WTILE(dram["w_fn"][d], 2 * HT))
            W["w_cmp"] = pp.tile([P, 2 * HT, H], WD, name="w_cmp", tag="w_cmp")
            nc.sync.dma_start(out=W["w_cmp"], in_=_WTILE(dram["w_cmp"], 2 * HT))
            W["w_o1"] = pp.tile([P, 2 * HT, H], WD, name="w_o1", tag="w_o1")
            nc.sync.dma_start(out=W["w_o1"], in_=_WTILE(dram["w_o1"], 2 * HT))
            W["w_o2"] = pp.tile([P, HT, 1], WD, name="w_o2", tag="w_o2")
            nc.sync.dma_start(out=W["w_o2"],
                              in_=dram["w_o2"].rearrange("(t p) o -> p t o", p=P))
        else:
            hoisted = True
            ids_sb = pro.tile([P, NL], F32, name="ids", tag="ids")
            nc.sync.dma_start(out=ids_sb, in_=dram["idsf"].partition_broadcast(P))
            emb_sb = W["emb"]
            w_hole_sb = W["w_hole"]
            xT_sb = pro.tile([P, HT, N], WD, name="xT", tag="xT")
            nc.sync.dma_start(out=xT_sb, in_=_WTILE(dram["xT"], HT))

        # persistent activations (per-rep lifetime)
        with tc.tile_pool(name="actp", bufs=1) as ap_pool:
            xw_sb = ap_pool.tile([P, HT, N], WD, name="xw", tag="xw")
            embG_sb = ap_pool.tile([P, HT, NL], WD, name="embG", tag="embG")

            # ---------- prologue: one-hot -> embG, xw ----------
            nhalf = (NL + 511) // 512
            eg_ps = [
                [ps.tile([P, 512], F32, name="mm", tag="mm", bufs=8)[:, : min(512, NL - 512 * h)]
                 for h in range(nhalf)]
                for _ in range(HT)
            ]
            for vt in range(VT):
                oh = ohp.tile([P, NL], WD, name="oh", tag="oh")
                nc.vector.tensor_scalar(
                    out=oh, in0=ids_sb, scalar1=W["iota"][:, vt: vt + 1],
                    scalar2=None, op0=ALU.is_equal,
                )
                for ht in range(HT):
                    for hf in range(nhalf):
                        _mm(
                            nc, eg_ps[ht][hf],
                            emb_sb[:, vt, ht * P:(ht + 1) * P],
                            oh[:, 512 * hf: 512 * hf + eg_ps[ht][hf].shape[-1]],
                            start=(vt == 0), stop=(vt == VT - 1),
                        )
            for ht in range(HT):
                for hf in range(nhalf):
                    w = eg_ps[ht][hf].shape[-1]
                    nc.scalar.activation(
                        out=embG_sb[:, ht, 512 * hf: 512 * hf + w],
                        in_=eg_ps[ht][hf], func=AF.Copy,
                    )

            accx = [ps.tile([P, 512], F32, name="mm", tag="mm", bufs=8)[:, :N]
                    for _ in range(HT)]
            for mt in range(HT):
                for kt in range(HT):
                    _mm(nc, accx[mt], w_hole_sb[:, kt, mt * P:(mt + 1) * P],
                        xT_sb[:, kt, :], start=(kt == 0), stop=(kt == HT - 1))
            for mt in range(HT):
                nc.scalar.activation(
                    out=xw_sb[:, mt, :], in_=accx[mt], func=AF.Identity,
                    bias=W["b_hole"][:, mt: mt + 1],
                )

            ddump("d_xw", xw_sb)
            ddump("d_embG", embG_sb)
            # ---------- main loop: leaf / RNN / streaming tree ----------
            h_prev = None
            pending = [None] * D
            root = [None]

            with (
                tc.tile_pool(name="leafp", bufs=3) as leafp,
                tc.tile_pool(name="lvl0", bufs=2) as lvl0p,
                tc.tile_pool(name="lvl1", bufs=2) as lvl1p,
                tc.tile_pool(name="lvl2", bufs=2) as lvl2p,
            ):
                lvlp = [lvl0p, lvl1p, lvl2p]

                def emit_fn_node(d, left, right):
                    if d == D - 1:
                        out_t = ap_pool.tile([P, HT, N], WD, name="root", tag="root")
                    else:
                        out_t = lvlp[d].tile([P, HT, N], WD, name=f"lvl{d}",
                                             tag=f"lvl{d}")
                    accs = [ps.tile([P, 512], F32, name="mm", tag="mm", bufs=8)[:, :N]
                            for _ in range(HT)]
                    for mt in range(HT):
                        for kt in range(2 * HT):
                            src = left if kt < HT else right
                            _mm(nc, accs[mt],
                                W["w_fn"][d][:, kt, mt * P:(mt + 1) * P],
                                src[:, kt % HT, :],
                                start=(kt == 0), stop=(kt == 2 * HT - 1))
                    for mt in range(HT):
                        nc.scalar.activation(
                            out=out_t[:, mt, :], in_=accs[mt],
                            func=AF.Relu, bias=W["b_fn"][:, d, mt: mt + 1],
                        )
                    return out_t

                def feed(d, node):
                    while True:
                        if pending[d] is None:
                            pending[d] = node
                            return
                        left = pending[d]
                        pending[d] = None
                        node = emit_fn_node(d, left, node)
                        if d == D - 1:
                            root[0] = node
                            return
                        d += 1

                for t in range(L):
                    # per-ht add+relu on DVE so each W_ih k-tile can start as
                    # soon as its ht slice is ready.  NEVER use gpsimd here:
                    # the GPSIMD engine has a huge real per-op cost (~24us/op
                    # measured on HW) that the cost model misses.
                    leaf_t = leafp.tile([P, HT, N], WD, name="leaf", tag="leaf")
                    for ht in range(HT):
                        egb = (
                            embG_sb[:, ht, :]
                            .rearrange("p (b l) -> p b l", l=L)[:, :, t]
                            .broadcast_to((P, NB, E))
                        )
                        nc.vector.tensor_tensor(
                            out=leaf_t[:, ht, :].rearrange("p (b e) -> p b e", e=E),
                            in0=xw_sb[:, ht, :].rearrange("p (b e) -> p b e", e=E),
                            in1=egb, op=ALU.add,
                        )
                        nc.vector.tensor_scalar(
                            out=leaf_t[:, ht, :], in0=leaf_t[:, ht, :],
                            scalar1=0.0, scalar2=None, op0=ALU.max,
                        )

                    # RNN step, two-pass (hide tanh under next W_ih matmuls)
                    h_t = ap_pool.tile([P, HT, N], WD, name="h", tag="h", bufs=2)
                    accr = [ps.tile([P, 512], F32, name="mm", tag="mm", bufs=8)[:, :N]
                            for _ in range(HT)]
                    only_ih = h_prev is None
                    for mt in range(HT):
                        for kt in range(HT):
                            _mm(nc, accr[mt],
                                W["w_ih"][:, kt, mt * P:(mt + 1) * P],
                                leaf_t[:, kt, :], start=(kt == 0),
                                stop=(only_ih and kt == HT - 1))
                    if not only_ih:
                        for mt in range(HT):
                            for kt in range(HT):
                                _mm(nc, accr[mt],
                                    W["w_hh"][:, kt, mt * P:(mt + 1) * P],
                                    h_prev[:, kt, :],
                                    start=False, stop=(kt == HT - 1))
                    for mt in range(HT):
                        nc.scalar.activation(
                            out=h_t[:, mt, :], in_=accr[mt],
                            func=AF.Tanh, bias=W["b_rnn"][:, mt: mt + 1],
                        )
                    h_prev = h_t
                    if t == 3:
                        ddump("d_leaf3", leaf_t)
                    if t == 0:
                        ddump("d_h0", h_t)

                    feed(0, leaf_t)

            ctx = h_prev
            rt = root[0]
            ddump("d_ctx", ctx)
            ddump("d_root", rt)

            # ---------- tail: cmp, E-max, output head ----------
            with tc.tile_pool(name="tail", bufs=1) as tp:
                cpd = tp.tile([P, 2 * HT, NB], WD, name="cpd", tag="cpd")
                for ht in range(HT):
                    nc.vector.reduce_max(
                        out=cpd[:, HT + ht, :],
                        in_=ctx[:, ht, :].rearrange("p (b e) -> p b e", e=E),
                        axis=mybir.AxisListType.X,
                    )

                cmp_sb = tp.tile([P, HT, N], WD, name="cmp", tag="cmp")
                accc = [ps.tile([P, 512], F32, name="mm", tag="mm", bufs=8)[:, :N]
                        for _ in range(HT)]
                for mt in range(HT):
                    for kt in range(2 * HT):
                        src = rt if kt < HT else ctx
                        _mm(nc, accc[mt],
                            W["w_cmp"][:, kt, mt * P:(mt + 1) * P],
                            src[:, kt % HT, :],
                            start=(kt == 0), stop=(kt == 2 * HT - 1))
                for mt in range(HT):
                    nc.scalar.activation(
                        out=cmp_sb[:, mt, :], in_=accc[mt],
                        func=AF.Relu, bias=W["b_cmp"][:, mt: mt + 1],
                    )

                ddump("d_cmp", cmp_sb)
                for ht in range(HT):
                    nc.vector.reduce_max(
                        out=cpd[:, ht, :],
                        in_=cmp_sb[:, ht, :].rearrange("p (b e) -> p b e", e=E),
                        axis=mybir.AxisListType.X,
                    )

                ddump("d_cpd", cpd)
                o1_sb = tp.tile([P, HT, NB], WD, name="o1", tag="o1")
                acco = [ps.tile([P, 512], F32, name="mm", tag="mm", bufs=8)[:, :NB]
                        for _ in range(HT)]
                # ctx-half k-tiles (ready early) first, cmp-half last, so the
                # o1 matmuls start before the cmp reduce_max ops finish
                korder = list(range(HT, 2 * HT)) + list(range(HT))
                for mt in range(HT):
                    for j, kt in enumerate(korder):
                        _mm(nc, acco[mt],
                            W["w_o1"][:, kt, mt * P:(mt + 1) * P],
                            cpd[:, kt, :], start=(j == 0),
                            stop=(j == 2 * HT - 1))
                for mt in range(HT):
                    nc.scalar.activation(
                        out=o1_sb[:, mt, :], in_=acco[mt],
                        func=AF.Relu, bias=W["b_o1"][:, mt: mt + 1],
                    )

                ddump("d_o1", o1_sb)
                acc2 = ps.tile([P, 512], F32, name="mm", tag="mm",
                               bufs=8)[0:1, :NB]
                for kt in range(HT):
                    _mm(nc, acc2, W["w_o2"][:, kt, :], o1_sb[:, kt, :],
                        start=(kt == 0), stop=(kt == HT - 1))
                ex = tp.tile([1, NB], F32, name="ex", tag="ex")
                nc.scalar.activation(out=ex, in_=acc2, func=AF.Exp,
                                     bias=W["b_o2"][0:1, 0:1])
                ex1 = tp.tile([1, NB], F32, name="ex1", tag="ex1")
                nc.vector.tensor_scalar(out=ex1, in0=ex, scalar1=1.0,
                                        scalar2=None, op0=ALU.add)
                sp = tp.tile([1, NB], F32, name="sp", tag="sp")
                nc.scalar.activation(out=sp, in_=ex1, func=AF.Ln)
                nc.sync.dma_start(out=val[None, :], in_=sp[0:1, :])


# ---------------------------------------------------------------------------
# host side
# ---------------------------------------------------------------------------

_NC_CACHE = {}


def _get_nc(NB=NB_FULL, zb=False):
    key = (NB, zb)
    if key not in _NC_CACHE:
        _NC_CACHE[key] = build_nc(NB, zb=zb)
    return _NC_CACHE[key]


def make_in_maps(inputs, NB=NB_FULL, ncores=NCORES, wdt="bf16"):
    bf16 = ml_dtypes.bfloat16 if wdt == "bf16" else np.float32
    x = np.asarray(inputs["x"], dtype=np.float32)
    leaf_ids = np.asarray(inputs["leaf_ids"])

    def wb(name):
        return np.ascontiguousarray(np.asarray(inputs[name], np.float32).astype(bf16))

    def bf(name):
        return np.ascontiguousarray(np.asarray(inputs[name], np.float32))

    shared = {
        "emb": wb("emb"), "w_hole": wb("W_hole"), "w_fn": wb("W_fn"),
        "w_ih": wb("W_ih"), "w_hh": wb("W_hh"), "w_cmp": wb("W_cmp"),
        "w_o1": wb("W_o1"), "w_o2": wb("W_o2"),
        "b_hole": bf("b_hole"), "b_fn": bf("b_fn"), "b_rnn": bf("b_rnn"),
        "b_cmp": bf("b_cmp"), "b_o1": bf("b_o1"), "b_o2": bf("b_o2"),
        "iota": np.ascontiguousarray(
            (np.arange(P)[:, None] + P * np.arange(VT)[None, :]).astype(np.float32)
        ),
    }
    in_maps = []
    for c in range(ncores):
        xs = x[c * NB:(c + 1) * NB].reshape(NB * E, H)
        ids = leaf_ids[c * NB:(c + 1) * NB].astype(np.float32).ravel()
        m = dict(shared)
        m["xT"] = np.ascontiguousarray(xs.T.astype(bf16))
        m["idsf"] = np.ascontiguousarray(ids)
        in_maps.append(m)
    return in_maps


def kernel(**inputs):
    from concourse.bass_utils import run_bass_kernel_spmd

    zb = all(
        not np.any(np.asarray(inputs[b]))
        for b in ("b_hole", "b_fn", "b_rnn", "b_cmp", "b_o1", "b_o2")
    )
    nc = _get_nc(NB_FULL, zb=zb)
    in_maps = make_in_maps(inputs)
    res = run_bass_kernel_spmd(nc, in_maps, list(range(NCORES))).results
    out = np.concatenate([np.asarray(res[c]["val"]) for c in range(NCORES)])
    return out.astype(np.float32)


# revision 16
# speedup vs baseline: 1.0295x; 1.0295x over previous
"""Trainium2 Bass kernel for nn_ListREPLValueHead (tree NN + RNN value head).

Strategy
--------
Data-parallel over the sketch batch B=512 across 8 NeuronCores (64 sketches,
N = 64*8 = 512 example-columns per core).  All weights replicated.

All activations live in *transposed* layout [H on partitions, batch on free]:
every GEMM is then `out' = W.T-matmul(act')` with the native weight matrix as
the stationary lhsT operand and activations as the moving rhs — no transposes
anywhere on device (x is pre-transposed on host; the embedding gather is done
as a one-hot matmul so the gather result is produced directly transposed).

The whole datapath is bf16 (PSUM accumulation fp32): same PE throughput as
fp32r (1 cycle/row) but half the DMA bytes / SBUF footprint, 2x DVE, and much
better behaved on HW inside PSUM accumulation groups.  End-to-end
quantization error ~9e-3 vs the 2e-2 gate.

DMA priority order matters: one serial DMA stream, so the prologue-critical
tensors (iota, ids, emb, w_hole, xT) are issued first and emb is split into 4
chunks so the one-hot GEMM starts after chunk 0 (~3us).  Bulk weights follow,
tail weights (w_cmp/w_o1) prefetch during the main loop.

Pipeline per core:
  1. one-hot(leaf_ids) [V, B*L]  (DVE is_equal vs iota)
  2. embG' = emb.T @ onehot   [H, B*L]   (PE, ACT evacuates)
  3. xw'  = W_hole.T @ x' + b_hole  [H, N]  (PE + ACT)
  4. t = 0..15:   leaf_t' = relu(xw' + embG'[:, :, t] bcast-e) (DVE add,
                         Pool relu)
                  h_t' = tanh(W_ih.T leaf_t' + W_hh.T h_{t-1}' + b)
                         (PE two-pass: all W_ih k-tiles for the 4 m-tiles
                         first, then W_hh — hides the tanh ACT latency of
                         step t under the W_ih matmuls of step t+1)
                  after odd t: fold tree nodes streaming (PE + ACT relu),
                         O(depth) memory
  5. cmp' = relu(W_cmp.T [root'; ctx'] + b_cmp)   (concat via K-slicing)
  6. compared' = max over E  -> [2H, 64]          (DVE strided reduce)
  7. val = softplus(W_o2.T relu(W_o1.T compared' + b_o1) + b_o2)
     as Ln(1+Exp(.)) on ACT.

build_nc(reps>1) repeats the body with weights hoisted out of the loop —
bench-only mode measuring the steady-state compute time via dispatch slope.
"""

import numpy as np
import ml_dtypes

import concourse.bass as bass
import concourse.mybir as mybir
import concourse.tile as tile
from concourse import bacc

F32 = mybir.dt.float32
BF16 = mybir.dt.bfloat16
AF = mybir.ActivationFunctionType
ALU = mybir.AluOpType

NCORES = 8
B, E, L, H, V, D = 512, 8, 16, 512, 512, 4
P = 128
HT = H // P          # 4 h-tiles
VT = V // P          # 4 v-tiles
NB_FULL = B // NCORES  # 64 sketches per core

_WTILE = lambda ap_, kt: ap_.rearrange("(kt p) m -> p kt m", p=P)  # noqa: E731


def _mm(nc, psum, lhsT, rhs, start, stop):
    nc.tensor.matmul(out=psum, lhsT=lhsT, rhs=rhs, start=start, stop=stop)


def build_nc(NB=NB_FULL, reps=1, dbg=False, wdt="bf16", zb=False):  # zb kept for compat, unused
    """Emit the per-core program. NB = sketches per core (64 full size).
    reps>1: benchmark mode — repeat the body, weights loaded once."""
    N = NB * E
    NL = NB * L

    WD = BF16 if wdt == "bf16" else mybir.dt.float32r
    nc = bacc.Bacc("TRN2", target_bir_lowering=False, debug=False)

    dram = {}
    dram["xT"] = nc.dram_tensor("xT", [H, N], WD, kind="ExternalInput").ap()
    dram["idsf"] = nc.dram_tensor("idsf", [NL], F32, kind="ExternalInput").ap()
    dram["iota"] = nc.dram_tensor("iota", [P, VT], F32, kind="ExternalInput").ap()
    dram["emb"] = nc.dram_tensor("emb", [V, H], WD, kind="ExternalInput").ap()
    dram["w_hole"] = nc.dram_tensor("w_hole", [H, H], WD, kind="ExternalInput").ap()
    dram["w_fn"] = nc.dram_tensor("w_fn", [D, 2 * H, H], WD, kind="ExternalInput").ap()
    dram["w_ih"] = nc.dram_tensor("w_ih", [H, H], WD, kind="ExternalInput").ap()
    dram["w_hh"] = nc.dram_tensor("w_hh", [H, H], WD, kind="ExternalInput").ap()
    dram["w_cmp"] = nc.dram_tensor("w_cmp", [2 * H, H], WD, kind="ExternalInput").ap()
    dram["w_o1"] = nc.dram_tensor("w_o1", [2 * H, H], WD, kind="ExternalInput").ap()
    dram["w_o2"] = nc.dram_tensor("w_o2", [H, 1], WD, kind="ExternalInput").ap()
    for bn, shp in [("b_hole", [H]), ("b_fn", [D, H]), ("b_rnn", [H]),
                    ("b_cmp", [H]), ("b_o1", [H]), ("b_o2", [1])]:
        dram[bn] = nc.dram_tensor(bn, shp, F32, kind="ExternalInput").ap()
    val = nc.dram_tensor("val", [NB], F32, kind="ExternalOutput").ap()
    dbg_outs = {}
    if dbg:
        for nm, shape in [
            ("d_xw", [P, HT, N]), ("d_embG", [P, HT, NL]),
            ("d_leaf3", [P, HT, N]), ("d_h0", [P, HT, N]),
            ("d_ctx", [P, HT, N]), ("d_root", [P, HT, N]),
            ("d_cmp", [P, HT, N]), ("d_cpd", [P, 2 * HT, NB]),
            ("d_o1", [P, HT, NB]),
        ]:
            dbg_outs[nm] = nc.dram_tensor(nm, shape, WD, kind="ExternalOutput").ap()

    with tile.TileContext(nc) as tc:
        with (
            tc.tile_pool(name="persist", bufs=1) as pp,
            tc.tile_pool(name="psum", bufs=8, space="PSUM") as ps,
        ):
            if reps == 1:
                _emit(nc, tc, NB, WD, dbg_outs, pp, ps, dram, val, W=None, zb=zb)
            else:
                W = _load_weights(nc, pp, WD, dram, hoist_inputs=True)
                for _ in range(reps):
                    _emit(nc, tc, NB, WD, {}, pp, ps, dram, val, W=W, zb=zb)

    nc.compile()
    return nc


def _load_bias(nc, pool, ap_, tag):
    t = pool.tile([P, HT], F32, tag=tag)
    nc.sync.dma_start(out=t, in_=ap_.rearrange("(t p) -> p t", p=P))
    return t


def _load_weights(nc, pp, WD, dram, hoist_inputs=False):
    """Load everything weight-like into the persistent pool (bench mode:
    emb/w_hole also persist so the rep body only re-loads ids/xT)."""
    W = {}
    W["iota"] = pp.tile([P, VT], F32, name="iota", tag="iota")
    nc.sync.dma_start(out=W["iota"], in_=dram["iota"])
    for bn in ("b_hole", "b_rnn", "b_cmp", "b_o1"):
        W[bn] = _load_bias(nc, pp, dram[bn], bn)
    W["b_fn"] = pp.tile([P, D, HT], F32, name="b_fn", tag="b_fn")
    nc.sync.dma_start(out=W["b_fn"],
                      in_=dram["b_fn"].rearrange("d (t p) -> p d t", p=P))
    W["b_o2"] = pp.tile([1, 1], F32, name="b_o2", tag="b_o2")
    nc.sync.dma_start(out=W["b_o2"], in_=dram["b_o2"][None, :])
    if hoist_inputs:
        W["emb"] = pp.tile([P, VT, H], WD, name="emb", tag="emb")
        for vt in range(VT):
            nc.sync.dma_start(out=W["emb"][:, vt, :],
                              in_=dram["emb"][vt * P:(vt + 1) * P, :])
        W["w_hole"] = pp.tile([P, HT, H], WD, name="w_hole", tag="w_hole")
        nc.sync.dma_start(out=W["w_hole"], in_=_WTILE(dram["w_hole"], HT))
    W["w_ih"] = pp.tile([P, HT, H], WD, name="w_ih", tag="w_ih")
    nc.sync.dma_start(out=W["w_ih"], in_=_WTILE(dram["w_ih"], HT))
    W["w_hh"] = pp.tile([P, HT, H], WD, name="w_hh", tag="w_hh")
    nc.sync.dma_start(out=W["w_hh"], in_=_WTILE(dram["w_hh"], HT))
    W["w_fn"] = [pp.tile([P, 2 * HT, H], WD, name=f"w_fn{d}", tag=f"w_fn{d}")
                 for d in range(D)]
    for d in range(D):
        nc.sync.dma_start(out=W["w_fn"][d], in_=_WTILE(dram["w_fn"][d], 2 * HT))
    W["w_cmp"] = pp.tile([P, 2 * HT, H], WD, name="w_cmp", tag="w_cmp")
    nc.sync.dma_start(out=W["w_cmp"], in_=_WTILE(dram["w_cmp"], 2 * HT))
    W["w_o1"] = pp.tile([P, 2 * HT, H], WD, name="w_o1", tag="w_o1")
    nc.sync.dma_start(out=W["w_o1"], in_=_WTILE(dram["w_o1"], 2 * HT))
    W["w_o2"] = pp.tile([P, HT, 1], WD, name="w_o2", tag="w_o2")
    nc.sync.dma_start(out=W["w_o2"],
                      in_=dram["w_o2"].rearrange("(t p) o -> p t o", p=P))
    return W


def _emit(nc, tc, NB, WD, dbg_outs, pp, ps, dram, val, W=None, zb=False):
    N = NB * E
    NL = NB * L

    def ddump(nm, t):
        if nm in dbg_outs:
            nc.sync.dma_start(out=dbg_outs[nm], in_=t)

    with (
        tc.tile_pool(name="pro", bufs=1) as pro,
        tc.tile_pool(name="ohp", bufs=2) as ohp,
    ):
        # ---------- per-call inputs + (reps==1) priority-ordered weights ----
        if W is None:
            # graded path: interleave weight DMAs in critical-path order
            hoisted = False
            W = {}
            W["iota"] = pp.tile([P, VT], F32, name="iota", tag="iota")
            nc.sync.dma_start(out=W["iota"], in_=dram["iota"])
            # xw operands first (xw GEMM runs before the one-hot GEMM);
            # xT chunked per k-tile so the first matmul starts ~2us in.
            # ids only feeds the DVE is_equal masks (~8us in), so it loads
            # after the xw operands.
            w_hole_sb = pro.tile([P, HT, H], WD, name="w_hole", tag="w_hole")
            nc.sync.dma_start(out=w_hole_sb, in_=_WTILE(dram["w_hole"], HT))
            xT_sb = pro.tile([P, HT, N], WD, name="xT", tag="xT")
            for kt in range(HT):
                nc.sync.dma_start(out=xT_sb[:, kt, :],
                                  in_=_WTILE(dram["xT"], HT)[:, kt, :])
            ids_sb = pro.tile([P, NL], F32, name="ids", tag="ids")
            nc.sync.dma_start(out=ids_sb, in_=dram["idsf"].partition_broadcast(P))
            emb_sb = pro.tile([P, VT, H], WD, name="emb", tag="emb")
            for vt in range(VT):
                nc.sync.dma_start(out=emb_sb[:, vt, :],
                                  in_=dram["emb"][vt * P:(vt + 1) * P, :])
            for bn in ("b_hole", "b_rnn", "b_cmp", "b_o1"):
                W[bn] = _load_bias(nc, pp, dram[bn], bn)
            W["b_fn"] = pp.tile([P, D, HT], F32, name="b_fn", tag="b_fn")
            nc.sync.dma_start(out=W["b_fn"],
                              in_=dram["b_fn"].rearrange("d (t p) -> p d t", p=P))
            W["b_o2"] = pp.tile([1, 1], F32, name="b_o2", tag="b_o2")
            nc.sync.dma_start(out=W["b_o2"], in_=dram["b_o2"][None, :])
            W["w_ih"] = pp.tile([P, HT, H], WD, name="w_ih", tag="w_ih")
            nc.sync.dma_start(out=W["w_ih"], in_=_WTILE(dram["w_ih"], HT))
            W["w_hh"] = pp.tile([P, HT, H], WD, name="w_hh", tag="w_hh")
            nc.sync.dma_start(out=W["w_hh"], in_=_WTILE(dram["w_hh"], HT))
            W["w_fn"] = [pp.tile([P, 2 * HT, H], WD, name=f"w_fn{d}",
                                 tag=f"w_fn{d}") for d in range(D)]
            for d in range(D):
                nc.sync.dma_start(out=W["w_fn"][d],
                                  in_=_WTILE(dram["w_fn"][d], 2 * HT))
            W["w_cmp"] = pp.tile([P, 2 * HT, H], WD, name="w_cmp", tag="w_cmp")
            nc.sync.dma_start(out=W["w_cmp"], in_=_WTILE(dram["w_cmp"], 2 * HT))
            W["w_o1"] = pp.tile([P, 2 * HT, H], WD, name="w_o1", tag="w_o1")
            nc.sync.dma_start(out=W["w_o1"], in_=_WTILE(dram["w_o1"], 2 * HT))
            W["w_o2"] = pp.tile([P, HT, 1], WD, name="w_o2", tag="w_o2")
            nc.sync.dma_start(out=W["w_o2"],
                              in_=dram["w_o2"].rearrange("(t p) o -> p t o", p=P))
        else:
            hoisted = True
            ids_sb = pro.tile([P, NL], F32, name="ids", tag="ids")
            nc.sync.dma_start(out=ids_sb, in_=dram["idsf"].partition_broadcast(P))
            emb_sb = W["emb"]
            w_hole_sb = W["w_hole"]
            xT_sb = pro.tile([P, HT, N], WD, name="xT", tag="xT")
            nc.sync.dma_start(out=xT_sb, in_=_WTILE(dram["xT"], HT))

        # persistent activations (per-rep lifetime)
        with tc.tile_pool(name="actp", bufs=1) as ap_pool:
            xw_sb = ap_pool.tile([P, HT, N], WD, name="xw", tag="xw")
            embG_sb = ap_pool.tile([P, HT, NL], WD, name="embG", tag="embG")

            # ---------- prologue: xw, then one-hot -> embG ----------
            # xw GEMM first (kt-outer: only needs xT chunk kt), so PE starts
            # ~2us in; the is_equal masks compute on DVE meanwhile.
            accx = [ps.tile([P, 512], F32, name="mm", tag="mm", bufs=8)[:, :N]
                    for _ in range(HT)]
            for kt in range(HT):
                for mt in range(HT):
                    _mm(nc, accx[mt], w_hole_sb[:, kt, mt * P:(mt + 1) * P],
                        xT_sb[:, kt, :], start=(kt == 0), stop=(kt == HT - 1))
            for mt in range(HT):
                nc.scalar.activation(
                    out=xw_sb[:, mt, :], in_=accx[mt], func=AF.Identity,
                    bias=W["b_hole"][:, mt: mt + 1],
                )

            # one-hot GEMM ht-outer: embG[ht] evacuates as soon as its 2
            # PSUM groups close, so the t=0 leaf chain (DVE) overlaps the
            # remaining one-hot work instead of waiting for all of it.
            nhalf = (NL + 511) // 512
            ohs = []
            for vt in range(VT):
                oh = ohp.tile([P, NL], WD, name="oh", tag="oh", bufs=VT)
                nc.vector.tensor_scalar(
                    out=oh, in0=ids_sb, scalar1=W["iota"][:, vt: vt + 1],
                    scalar2=None, op0=ALU.is_equal,
                )
                ohs.append(oh)
            for ht in range(HT):
                eg = [ps.tile([P, 512], F32, name="mm", tag="mm", bufs=8)[:, : min(512, NL - 512 * h)]
                      for h in range(nhalf)]
                for vt in range(VT):
                    for hf in range(nhalf):
                        _mm(
                            nc, eg[hf],
                            emb_sb[:, vt, ht * P:(ht + 1) * P],
                            ohs[vt][:, 512 * hf: 512 * hf + eg[hf].shape[-1]],
                            start=(vt == 0), stop=(vt == VT - 1),
                        )
                for hf in range(nhalf):
                    w = eg[hf].shape[-1]
                    nc.scalar.activation(
                        out=embG_sb[:, ht, 512 * hf: 512 * hf + w],
                        in_=eg[hf], func=AF.Copy,
                    )

            ddump("d_xw", xw_sb)
            ddump("d_embG", embG_sb)
            # ---------- main loop: leaf / RNN / streaming tree ----------
            h_prev = None
            pending = [None] * D
            root = [None]

            with (
                tc.tile_pool(name="leafp", bufs=3) as leafp,
                tc.tile_pool(name="lvl0", bufs=2) as lvl0p,
                tc.tile_pool(name="lvl1", bufs=2) as lvl1p,
                tc.tile_pool(name="lvl2", bufs=2) as lvl2p,
            ):
                lvlp = [lvl0p, lvl1p, lvl2p]

                def emit_fn_node(d, left, right):
                    if d == D - 1:
                        out_t = ap_pool.tile([P, HT, N], WD, name="root", tag="root")
                    else:
                        out_t = lvlp[d].tile([P, HT, N], WD, name=f"lvl{d}",
                                             tag=f"lvl{d}")
                    accs = [ps.tile([P, 512], F32, name="mm", tag="mm", bufs=8)[:, :N]
                            for _ in range(HT)]
                    for mt in range(HT):
                        for kt in range(2 * HT):
                            src = left if kt < HT else right
                            _mm(nc, accs[mt],
                                W["w_fn"][d][:, kt, mt * P:(mt + 1) * P],
                                src[:, kt % HT, :],
                                start=(kt == 0), stop=(kt == 2 * HT - 1))
                    for mt in range(HT):
                        nc.scalar.activation(
                            out=out_t[:, mt, :], in_=accs[mt],
                            func=AF.Relu, bias=W["b_fn"][:, d, mt: mt + 1],
                        )
                    return out_t

                def feed(d, node):
                    while True:
                        if pending[d] is None:
                            pending[d] = node
                            return
                        left = pending[d]
                        pending[d] = None
                        node = emit_fn_node(d, left, node)
                        if d == D - 1:
                            root[0] = node
                            return
                        d += 1

                for t in range(L):
                    # per-ht add+relu on DVE so each W_ih k-tile can start as
                    # soon as its ht slice is ready.  NEVER use gpsimd here:
                    # the GPSIMD engine has a huge real per-op cost (~24us/op
                    # measured on HW) that the cost model misses.
                    leaf_t = leafp.tile([P, HT, N], WD, name="leaf", tag="leaf")
                    for ht in range(HT):
                        egb = (
                            embG_sb[:, ht, :]
                            .rearrange("p (b l) -> p b l", l=L)[:, :, t]
                            .broadcast_to((P, NB, E))
                        )
                        nc.vector.tensor_tensor(
                            out=leaf_t[:, ht, :].rearrange("p (b e) -> p b e", e=E),
                            in0=xw_sb[:, ht, :].rearrange("p (b e) -> p b e", e=E),
                            in1=egb, op=ALU.add,
                        )
                        nc.vector.tensor_scalar(
                            out=leaf_t[:, ht, :], in0=leaf_t[:, ht, :],
                            scalar1=0.0, scalar2=None, op0=ALU.max,
                        )

                    # RNN step, two-pass (hide tanh under next W_ih matmuls)
                    h_t = ap_pool.tile([P, HT, N], WD, name="h", tag="h", bufs=2)
                    accr = [ps.tile([P, 512], F32, name="mm", tag="mm", bufs=8)[:, :N]
                            for _ in range(HT)]
                    only_ih = h_prev is None
                    for mt in range(HT):
                        for kt in range(HT):
                            _mm(nc, accr[mt],
                                W["w_ih"][:, kt, mt * P:(mt + 1) * P],
                                leaf_t[:, kt, :], start=(kt == 0),
                                stop=(only_ih and kt == HT - 1))
                    if not only_ih:
                        for mt in range(HT):
                            for kt in range(HT):
                                _mm(nc, accr[mt],
                                    W["w_hh"][:, kt, mt * P:(mt + 1) * P],
                                    h_prev[:, kt, :],
                                    start=False, stop=(kt == HT - 1))
                    for mt in range(HT):
                        nc.scalar.activation(
                            out=h_t[:, mt, :], in_=accr[mt],
                            func=AF.Tanh, bias=W["b_rnn"][:, mt: mt + 1],
                        )
                    h_prev = h_t
                    if t == 3:
                        ddump("d_leaf3", leaf_t)
                    if t == 0:
                        ddump("d_h0", h_t)

                    feed(0, leaf_t)

            ctx = h_prev
            rt = root[0]
            ddump("d_ctx", ctx)
            ddump("d_root", rt)

            # ---------- tail: cmp, E-max, output head ----------
            with tc.tile_pool(name="tail", bufs=1) as tp:
                cpd = tp.tile([P, 2 * HT, NB], WD, name="cpd", tag="cpd")
                for ht in range(HT):
                    nc.vector.reduce_max(
                        out=cpd[:, HT + ht, :],
                        in_=ctx[:, ht, :].rearrange("p (b e) -> p b e", e=E),
                        axis=mybir.AxisListType.X,
                    )

                cmp_sb = tp.tile([P, HT, N], WD, name="cmp", tag="cmp")
                accc = [ps.tile([P, 512], F32, name="mm", tag="mm", bufs=8)[:, :N]
                        for _ in range(HT)]
                for mt in range(HT):
                    for kt in range(2 * HT):
                        src = rt if kt < HT else ctx
                        _mm(nc, accc[mt],
                            W["w_cmp"][:, kt, mt * P:(mt + 1) * P],
                            src[:, kt % HT, :],
                            start=(kt == 0), stop=(kt == 2 * HT - 1))
                for mt in range(HT):
                    nc.scalar.activation(
                        out=cmp_sb[:, mt, :], in_=accc[mt],
                        func=AF.Relu, bias=W["b_cmp"][:, mt: mt + 1],
                    )

                ddump("d_cmp", cmp_sb)
                for ht in range(HT):
                    nc.vector.reduce_max(
                        out=cpd[:, ht, :],
                        in_=cmp_sb[:, ht, :].rearrange("p (b e) -> p b e", e=E),
                        axis=mybir.AxisListType.X,
                    )

                ddump("d_cpd", cpd)
                o1_sb = tp.tile([P, HT, NB], WD, name="o1", tag="o1")
                acco = [ps.tile([P, 512], F32, name="mm", tag="mm", bufs=8)[:, :NB]
                        for _ in range(HT)]
                # ctx-half k-tiles (ready early) first, cmp-half last, so the
                # o1 matmuls start before the cmp reduce_max ops finish
                korder = list(range(HT, 2 * HT)) + list(range(HT))
                for mt in range(HT):
                    for j, kt in enumerate(korder):
                        _mm(nc, acco[mt],
                            W["w_o1"][:, kt, mt * P:(mt + 1) * P],
                            cpd[:, kt, :], start=(j == 0),
                            stop=(j == 2 * HT - 1))
                for mt in range(HT):
                    nc.scalar.activation(
                        out=o1_sb[:, mt, :], in_=acco[mt],
                        func=AF.Relu, bias=W["b_o1"][:, mt: mt + 1],
                    )

                ddump("d_o1", o1_sb)
                acc2 = ps.tile([P, 512], F32, name="mm", tag="mm",
                               bufs=8)[0:1, :NB]
                for kt in range(HT):
                    _mm(nc, acc2, W["w_o2"][:, kt, :], o1_sb[:, kt, :],
                        start=(kt == 0), stop=(kt == HT - 1))
                ex = tp.tile([1, NB], F32, name="ex", tag="ex")
                nc.scalar.activation(out=ex, in_=acc2, func=AF.Exp,
                                     bias=W["b_o2"][0:1, 0:1])
                ex1 = tp.tile([1, NB], F32, name="ex1", tag="ex1")
                nc.vector.tensor_scalar(out=ex1, in0=ex, scalar1=1.0,
                                        scalar2=None, op0=ALU.add)
                sp = tp.tile([1, NB], F32, name="sp", tag="sp")
                nc.scalar.activation(out=sp, in_=ex1, func=AF.Ln)
                nc.sync.dma_start(out=val[None, :], in_=sp[0:1, :])


# ---------------------------------------------------------------------------
# host side
# ---------------------------------------------------------------------------

_NC_CACHE = {}


def _get_nc(NB=NB_FULL, zb=False):
    key = (NB, zb)
    if key not in _NC_CACHE:
        _NC_CACHE[key] = build_nc(NB, zb=zb)
    return _NC_CACHE[key]


def make_in_maps(inputs, NB=NB_FULL, ncores=NCORES, wdt="bf16"):
    bf16 = ml_dtypes.bfloat16 if wdt == "bf16" else np.float32
    x = np.asarray(inputs["x"], dtype=np.float32)
    leaf_ids = np.asarray(inputs["leaf_ids"])

    def wb(name):
        return np.ascontiguousarray(np.asarray(inputs[name], np.float32).astype(bf16))

    def bf(name):
        return np.ascontiguousarray(np.asarray(inputs[name], np.float32))

    shared = {
        "emb": wb("emb"), "w_hole": wb("W_hole"), "w_fn": wb("W_fn"),
        "w_ih": wb("W_ih"), "w_hh": wb("W_hh"), "w_cmp": wb("W_cmp"),
        "w_o1": wb("W_o1"), "w_o2": wb("W_o2"),
        "b_hole": bf("b_hole"), "b_fn": bf("b_fn"), "b_rnn": bf("b_rnn"),
        "b_cmp": bf("b_cmp"), "b_o1": bf("b_o1"), "b_o2": bf("b_o2"),
        "iota": np.ascontiguousarray(
            (np.arange(P)[:, None] + P * np.arange(VT)[None, :]).astype(np.float32)
        ),
    }
    in_maps = []
    for c in range(ncores):
        xs = x[c * NB:(c + 1) * NB].reshape(NB * E, H)
        ids = leaf_ids[c * NB:(c + 1) * NB].astype(np.float32).ravel()
        m = dict(shared)
        m["xT"] = np.ascontiguousarray(xs.T.astype(bf16))
        m["idsf"] = np.ascontiguousarray(ids)
        in_maps.append(m)
    return in_maps


def kernel(**inputs):
    from concourse.bass_utils import run_bass_kernel_spmd

    zb = all(
        not np.any(np.asarray(inputs[b]))
        for b in ("b_hole", "b_fn", "b_rnn", "b_cmp", "b_o1", "b_o2")
    )
    nc = _get_nc(NB_FULL, zb=zb)
    in_maps = make_in_maps(inputs)
    res = run_bass_kernel_spmd(nc, in_maps, list(range(NCORES))).results
    out = np.concatenate([np.asarray(res[c]["val"]) for c in range(NCORES)])
    return out.astype(np.float32)


# revision 19
# speedup vs baseline: 1.0803x; 1.0493x over previous
"""Trainium2 Bass kernel for nn_ListREPLValueHead (tree NN + RNN value head).

Strategy
--------
Data-parallel over the sketch batch B=512 across 8 NeuronCores (64 sketches,
N = 64*8 = 512 example-columns per core).  All weights replicated.

All activations live in *transposed* layout [H on partitions, batch on free]:
every GEMM is then `out' = W.T-matmul(act')` with the native weight matrix as
the stationary lhsT operand and activations as the moving rhs — no transposes
anywhere on device (x is pre-transposed on host; the embedding gather is done
as a one-hot matmul so the gather result is produced directly transposed).

The whole datapath is bf16 (PSUM accumulation fp32): same PE throughput as
fp32r (1 cycle/row) but half the DMA bytes / SBUF footprint, 2x DVE, and much
better behaved on HW inside PSUM accumulation groups.  End-to-end
quantization error ~9e-3 vs the 2e-2 gate.

DMA priority order matters: one serial DMA stream, so the prologue-critical
tensors (iota, ids, emb, w_hole, xT) are issued first and emb is split into 4
chunks so the one-hot GEMM starts after chunk 0 (~3us).  Bulk weights follow,
tail weights (w_cmp/w_o1) prefetch during the main loop.

Pipeline per core:
  1. one-hot(leaf_ids) [V, B*L]  (DVE is_equal vs iota)
  2. embG' = emb.T @ onehot   [H, B*L]   (PE, ACT evacuates)
  3. xw'  = W_hole.T @ x' + b_hole  [H, N]  (PE + ACT)
  4. t = 0..15:   leaf_t' = relu(xw' + embG'[:, :, t] bcast-e) (DVE add,
                         Pool relu)
                  h_t' = tanh(W_ih.T leaf_t' + W_hh.T h_{t-1}' + b)
                         (PE two-pass: all W_ih k-tiles for the 4 m-tiles
                         first, then W_hh — hides the tanh ACT latency of
                         step t under the W_ih matmuls of step t+1)
                  after odd t: fold tree nodes streaming (PE + ACT relu),
                         O(depth) memory
  5. cmp' = relu(W_cmp.T [root'; ctx'] + b_cmp)   (concat via K-slicing)
  6. compared' = max over E  -> [2H, 64]          (DVE strided reduce)
  7. val = softplus(W_o2.T relu(W_o1.T compared' + b_o1) + b_o2)
     as Ln(1+Exp(.)) on ACT.

build_nc(reps>1) repeats the body with weights hoisted out of the loop —
bench-only mode measuring the steady-state compute time via dispatch slope.
"""

import numpy as np
import ml_dtypes

import concourse.bass as bass
import concourse.mybir as mybir
import concourse.tile as tile
from concourse import bacc

F32 = mybir.dt.float32
BF16 = mybir.dt.bfloat16
AF = mybir.ActivationFunctionType
ALU = mybir.AluOpType

NCORES = 8
B, E, L, H, V, D = 512, 8, 16, 512, 512, 4
P = 128
HT = H // P          # 4 h-tiles
VT = V // P          # 4 v-tiles
NB_FULL = B // NCORES  # 64 sketches per core

_WTILE = lambda ap_, kt: ap_.rearrange("(kt p) m -> p kt m", p=P)  # noqa: E731


def _mm(nc, psum, lhsT, rhs, start, stop):
    nc.tensor.matmul(out=psum, lhsT=lhsT, rhs=rhs, start=start, stop=stop)


def build_nc(NB=NB_FULL, reps=1, dbg=False, wdt="bf16", zb=False):  # zb kept for compat, unused
    """Emit the per-core program. NB = sketches per core (64 full size).
    reps>1: benchmark mode — repeat the body, weights loaded once."""
    N = NB * E
    NL = NB * L

    WD = BF16 if wdt == "bf16" else mybir.dt.float32r
    nc = bacc.Bacc("TRN2", target_bir_lowering=False, debug=False)

    dram = {}
    dram["xT"] = nc.dram_tensor("xT", [H, N], WD, kind="ExternalInput").ap()
    dram["idsf"] = nc.dram_tensor("idsf", [NL], F32, kind="ExternalInput").ap()
    dram["iota"] = nc.dram_tensor("iota", [P, VT], F32, kind="ExternalInput").ap()
    dram["emb"] = nc.dram_tensor("emb", [V, H], WD, kind="ExternalInput").ap()
    dram["w_hole"] = nc.dram_tensor("w_hole", [H, H], WD, kind="ExternalInput").ap()
    dram["w_fn"] = nc.dram_tensor("w_fn", [D, 2 * H, H], WD, kind="ExternalInput").ap()
    dram["w_ih"] = nc.dram_tensor("w_ih", [H, H], WD, kind="ExternalInput").ap()
    dram["w_hh"] = nc.dram_tensor("w_hh", [H, H], WD, kind="ExternalInput").ap()
    dram["w_cmp"] = nc.dram_tensor("w_cmp", [2 * H, H], WD, kind="ExternalInput").ap()
    dram["w_o1"] = nc.dram_tensor("w_o1", [2 * H, H], WD, kind="ExternalInput").ap()
    dram["w_o2"] = nc.dram_tensor("w_o2", [H, 1], WD, kind="ExternalInput").ap()
    for bn, shp in [("b_hole", [H]), ("b_fn", [D, H]), ("b_rnn", [H]),
                    ("b_cmp", [H]), ("b_o1", [H]), ("b_o2", [1])]:
        dram[bn] = nc.dram_tensor(bn, shp, F32, kind="ExternalInput").ap()
    val = nc.dram_tensor("val", [NB], F32, kind="ExternalOutput").ap()
    dbg_outs = {}
    if dbg:
        for nm, shape in [
            ("d_xw", [P, HT, N]), ("d_embG", [P, HT, NL]),
            ("d_leaf3", [P, HT, N]), ("d_h0", [P, HT, N]),
            ("d_ctx", [P, HT, N]), ("d_root", [P, HT, N]),
            ("d_cmp", [P, HT, N]), ("d_cpd", [P, 2 * HT, NB]),
            ("d_o1", [P, HT, NB]),
        ]:
            dbg_outs[nm] = nc.dram_tensor(nm, shape, WD, kind="ExternalOutput").ap()

    with tile.TileContext(nc) as tc:
        with (
            tc.tile_pool(name="persist", bufs=1) as pp,
            tc.tile_pool(name="psum", bufs=8, space="PSUM") as ps,
        ):
            if reps == 1:
                _emit(nc, tc, NB, WD, dbg_outs, pp, ps, dram, val, W=None, zb=zb)
            else:
                W = _load_weights(nc, pp, WD, dram, hoist_inputs=True)
                for _ in range(reps):
                    _emit(nc, tc, NB, WD, {}, pp, ps, dram, val, W=W, zb=zb)

    nc.compile()
    return nc


def _load_bias(nc, pool, ap_, tag):
    t = pool.tile([P, HT], F32, tag=tag)
    nc.sync.dma_start(out=t, in_=ap_.rearrange("(t p) -> p t", p=P))
    return t


def _load_weights(nc, pp, WD, dram, hoist_inputs=False):
    """Load everything weight-like into the persistent pool (bench mode:
    emb/w_hole also persist so the rep body only re-loads ids/xT)."""
    W = {}
    W["iota"] = pp.tile([P, VT], F32, name="iota", tag="iota")
    nc.sync.dma_start(out=W["iota"], in_=dram["iota"])
    for bn in ("b_hole", "b_rnn", "b_cmp", "b_o1"):
        W[bn] = _load_bias(nc, pp, dram[bn], bn)
    W["b_fn"] = pp.tile([P, D, HT], F32, name="b_fn", tag="b_fn")
    nc.sync.dma_start(out=W["b_fn"],
                      in_=dram["b_fn"].rearrange("d (t p) -> p d t", p=P))
    W["b_o2"] = pp.tile([1, 1], F32, name="b_o2", tag="b_o2")
    nc.sync.dma_start(out=W["b_o2"], in_=dram["b_o2"][None, :])
    if hoist_inputs:
        W["emb"] = pp.tile([P, VT, H], WD, name="emb", tag="emb")
        for vt in range(VT):
            nc.sync.dma_start(out=W["emb"][:, vt, :],
                              in_=dram["emb"][vt * P:(vt + 1) * P, :])
        W["w_hole"] = pp.tile([P, HT, H], WD, name="w_hole", tag="w_hole")
        nc.sync.dma_start(out=W["w_hole"], in_=_WTILE(dram["w_hole"], HT))
    W["w_ih"] = pp.tile([P, HT, H], WD, name="w_ih", tag="w_ih")
    nc.sync.dma_start(out=W["w_ih"], in_=_WTILE(dram["w_ih"], HT))
    W["w_hh"] = pp.tile([P, HT, H], WD, name="w_hh", tag="w_hh")
    nc.sync.dma_start(out=W["w_hh"], in_=_WTILE(dram["w_hh"], HT))
    W["w_fn"] = [pp.tile([P, 2 * HT, H], WD, name=f"w_fn{d}", tag=f"w_fn{d}")
                 for d in range(D)]
    for d in range(D):
        nc.sync.dma_start(out=W["w_fn"][d], in_=_WTILE(dram["w_fn"][d], 2 * HT))
    W["w_cmp"] = pp.tile([P, 2 * HT, H], WD, name="w_cmp", tag="w_cmp")
    nc.sync.dma_start(out=W["w_cmp"], in_=_WTILE(dram["w_cmp"], 2 * HT))
    W["w_o1"] = pp.tile([P, 2 * HT, H], WD, name="w_o1", tag="w_o1")
    nc.sync.dma_start(out=W["w_o1"], in_=_WTILE(dram["w_o1"], 2 * HT))
    W["w_o2"] = pp.tile([P, HT, 1], WD, name="w_o2", tag="w_o2")
    nc.sync.dma_start(out=W["w_o2"],
                      in_=dram["w_o2"].rearrange("(t p) o -> p t o", p=P))
    return W


def _emit(nc, tc, NB, WD, dbg_outs, pp, ps, dram, val, W=None, zb=False):
    N = NB * E
    NL = NB * L

    def ddump(nm, t):
        if nm in dbg_outs:
            nc.sync.dma_start(out=dbg_outs[nm], in_=t)

    with (
        tc.tile_pool(name="pro", bufs=1) as pro,
        tc.tile_pool(name="ohp", bufs=2) as ohp,
    ):
        # ---------- per-call inputs + (reps==1) priority-ordered weights ----
        if W is None:
            # graded path: interleave weight DMAs in critical-path order
            hoisted = False
            W = {}
            W["iota"] = pp.tile([P, VT], F32, name="iota", tag="iota")
            nc.sync.dma_start(out=W["iota"], in_=dram["iota"])
            # xw operands first (xw GEMM runs before the one-hot GEMM);
            # xT chunked per k-tile so the first matmul starts ~2us in.
            # ids only feeds the DVE is_equal masks (~8us in), so it loads
            # after the xw operands.
            w_hole_sb = pro.tile([P, HT, H], WD, name="w_hole", tag="w_hole")
            xT_sb = pro.tile([P, HT, N], WD, name="xT", tag="xT")
            for kt in range(HT):
                # interleave the two per-k-tile chunk streams so the kt=0
                # matmul's operands both arrive first
                nc.sync.dma_start(out=w_hole_sb[:, kt, :],
                                  in_=_WTILE(dram["w_hole"], HT)[:, kt, :])
                nc.sync.dma_start(out=xT_sb[:, kt, :],
                                  in_=_WTILE(dram["xT"], HT)[:, kt, :])
            ids_sb = pro.tile([P, NL], F32, name="ids", tag="ids")
            nc.sync.dma_start(out=ids_sb, in_=dram["idsf"].partition_broadcast(P))
            emb_sb = pro.tile([P, VT, H], WD, name="emb", tag="emb")
            for vt in range(VT):
                nc.sync.dma_start(out=emb_sb[:, vt, :],
                                  in_=dram["emb"][vt * P:(vt + 1) * P, :])
            for bn in ("b_hole", "b_rnn", "b_cmp", "b_o1"):
                W[bn] = _load_bias(nc, pp, dram[bn], bn)
            W["b_fn"] = pp.tile([P, D, HT], F32, name="b_fn", tag="b_fn")
            nc.sync.dma_start(out=W["b_fn"],
                              in_=dram["b_fn"].rearrange("d (t p) -> p d t", p=P))
            W["b_o2"] = pp.tile([1, 1], F32, name="b_o2", tag="b_o2")
            nc.sync.dma_start(out=W["b_o2"], in_=dram["b_o2"][None, :])
            W["w_ih"] = pp.tile([P, HT, H], WD, name="w_ih", tag="w_ih")
            nc.sync.dma_start(out=W["w_ih"], in_=_WTILE(dram["w_ih"], HT))
            W["w_hh"] = pp.tile([P, HT, H], WD, name="w_hh", tag="w_hh")
            nc.sync.dma_start(out=W["w_hh"], in_=_WTILE(dram["w_hh"], HT))
            W["w_fn"] = [pp.tile([P, 2 * HT, H], WD, name=f"w_fn{d}",
                                 tag=f"w_fn{d}") for d in range(D)]
            for d in range(D):
                nc.sync.dma_start(out=W["w_fn"][d],
                                  in_=_WTILE(dram["w_fn"][d], 2 * HT))
            W["w_cmp"] = pp.tile([P, 2 * HT, H], WD, name="w_cmp", tag="w_cmp")
            nc.sync.dma_start(out=W["w_cmp"], in_=_WTILE(dram["w_cmp"], 2 * HT))
            W["w_o1"] = pp.tile([P, 2 * HT, H], WD, name="w_o1", tag="w_o1")
            nc.sync.dma_start(out=W["w_o1"], in_=_WTILE(dram["w_o1"], 2 * HT))
            W["w_o2"] = pp.tile([P, HT, 1], WD, name="w_o2", tag="w_o2")
            nc.sync.dma_start(out=W["w_o2"],
                              in_=dram["w_o2"].rearrange("(t p) o -> p t o", p=P))
        else:
            hoisted = True
            ids_sb = pro.tile([P, NL], F32, name="ids", tag="ids")
            nc.sync.dma_start(out=ids_sb, in_=dram["idsf"].partition_broadcast(P))
            emb_sb = W["emb"]
            w_hole_sb = W["w_hole"]
            xT_sb = pro.tile([P, HT, N], WD, name="xT", tag="xT")
            nc.sync.dma_start(out=xT_sb, in_=_WTILE(dram["xT"], HT))

        # persistent activations (per-rep lifetime)
        with tc.tile_pool(name="actp", bufs=1) as ap_pool:
            xw_sb = ap_pool.tile([P, HT, N], WD, name="xw", tag="xw")
            embG_sb = ap_pool.tile([P, HT, NL], WD, name="embG", tag="embG")

            # ---------- prologue: xw, then one-hot -> embG ----------
            # xw GEMM first (kt-outer: only needs xT chunk kt), so PE starts
            # ~2us in; the is_equal masks compute on DVE meanwhile.
            accx = [ps.tile([P, 512], F32, name="mm", tag="mm", bufs=8)[:, :N]
                    for _ in range(HT)]
            for kt in range(HT):
                for mt in range(HT):
                    _mm(nc, accx[mt], w_hole_sb[:, kt, mt * P:(mt + 1) * P],
                        xT_sb[:, kt, :], start=(kt == 0), stop=(kt == HT - 1))
            for mt in range(HT):
                nc.scalar.activation(
                    out=xw_sb[:, mt, :], in_=accx[mt], func=AF.Identity,
                    bias=W["b_hole"][:, mt: mt + 1],
                )

            # one-hot GEMM ht-outer: embG[ht] evacuates as soon as its 2
            # PSUM groups close, so the t=0 leaf chain (DVE) overlaps the
            # remaining one-hot work instead of waiting for all of it.
            nhalf = (NL + 511) // 512
            ohs = []
            for vt in range(VT):
                oh = ohp.tile([P, NL], WD, name="oh", tag="oh", bufs=VT)
                nc.vector.tensor_scalar(
                    out=oh, in0=ids_sb, scalar1=W["iota"][:, vt: vt + 1],
                    scalar2=None, op0=ALU.is_equal,
                )
                ohs.append(oh)
            for ht in range(HT):
                eg = [ps.tile([P, 512], F32, name="mm", tag="mm", bufs=8)[:, : min(512, NL - 512 * h)]
                      for h in range(nhalf)]
                for vt in range(VT):
                    for hf in range(nhalf):
                        _mm(
                            nc, eg[hf],
                            emb_sb[:, vt, ht * P:(ht + 1) * P],
                            ohs[vt][:, 512 * hf: 512 * hf + eg[hf].shape[-1]],
                            start=(vt == 0), stop=(vt == VT - 1),
                        )
                for hf in range(nhalf):
                    w = eg[hf].shape[-1]
                    nc.scalar.activation(
                        out=embG_sb[:, ht, 512 * hf: 512 * hf + w],
                        in_=eg[hf], func=AF.Copy,
                    )

            ddump("d_xw", xw_sb)
            ddump("d_embG", embG_sb)
            # ---------- main loop: leaf / RNN / streaming tree ----------
            h_prev = None
            pending = [None] * D
            root = [None]

            with (
                tc.tile_pool(name="leafp", bufs=3) as leafp,
                tc.tile_pool(name="lvl0", bufs=2) as lvl0p,
                tc.tile_pool(name="lvl1", bufs=2) as lvl1p,
                tc.tile_pool(name="lvl2", bufs=2) as lvl2p,
            ):
                lvlp = [lvl0p, lvl1p, lvl2p]

                def emit_fn_node(d, left, right):
                    if d == D - 1:
                        out_t = ap_pool.tile([P, HT, N], WD, name="root", tag="root")
                    else:
                        out_t = lvlp[d].tile([P, HT, N], WD, name=f"lvl{d}",
                                             tag=f"lvl{d}")
                    accs = [ps.tile([P, 512], F32, name="mm", tag="mm", bufs=8)[:, :N]
                            for _ in range(HT)]
                    for mt in range(HT):
                        for kt in range(2 * HT):
                            src = left if kt < HT else right
                            _mm(nc, accs[mt],
                                W["w_fn"][d][:, kt, mt * P:(mt + 1) * P],
                                src[:, kt % HT, :],
                                start=(kt == 0), stop=(kt == 2 * HT - 1))
                    for mt in range(HT):
                        nc.scalar.activation(
                            out=out_t[:, mt, :], in_=accs[mt],
                            func=AF.Relu, bias=W["b_fn"][:, d, mt: mt + 1],
                        )
                    return out_t

                def feed(d, node):
                    while True:
                        if pending[d] is None:
                            pending[d] = node
                            return
                        left = pending[d]
                        pending[d] = None
                        node = emit_fn_node(d, left, node)
                        if d == D - 1:
                            root[0] = node
                            return
                        d += 1

                for t in range(L):
                    # per-ht add+relu on DVE so each W_ih k-tile can start as
                    # soon as its ht slice is ready.  NEVER use gpsimd here:
                    # the GPSIMD engine has a huge real per-op cost (~24us/op
                    # measured on HW) that the cost model misses.
                    leaf_t = leafp.tile([P, HT, N], WD, name="leaf", tag="leaf")
                    for ht in range(HT):
                        egb = (
                            embG_sb[:, ht, :]
                            .rearrange("p (b l) -> p b l", l=L)[:, :, t]
                            .broadcast_to((P, NB, E))
                        )
                        nc.vector.tensor_tensor(
                            out=leaf_t[:, ht, :].rearrange("p (b e) -> p b e", e=E),
                            in0=xw_sb[:, ht, :].rearrange("p (b e) -> p b e", e=E),
                            in1=egb, op=ALU.add,
                        )
                        nc.vector.tensor_scalar(
                            out=leaf_t[:, ht, :], in0=leaf_t[:, ht, :],
                            scalar1=0.0, scalar2=None, op0=ALU.max,
                        )

                    # RNN step, two-pass (hide tanh under next W_ih matmuls)
                    h_t = ap_pool.tile([P, HT, N], WD, name="h", tag="h", bufs=2)
                    accr = [ps.tile([P, 512], F32, name="mm", tag="mm", bufs=8)[:, :N]
                            for _ in range(HT)]
                    only_ih = h_prev is None
                    for mt in range(HT):
                        for kt in range(HT):
                            _mm(nc, accr[mt],
                                W["w_ih"][:, kt, mt * P:(mt + 1) * P],
                                leaf_t[:, kt, :], start=(kt == 0),
                                stop=(only_ih and kt == HT - 1))
                    if not only_ih:
                        for mt in range(HT):
                            for kt in range(HT):
                                _mm(nc, accr[mt],
                                    W["w_hh"][:, kt, mt * P:(mt + 1) * P],
                                    h_prev[:, kt, :],
                                    start=False, stop=(kt == HT - 1))
                    for mt in range(HT):
                        nc.scalar.activation(
                            out=h_t[:, mt, :], in_=accr[mt],
                            func=AF.Tanh, bias=W["b_rnn"][:, mt: mt + 1],
                        )
                    h_prev = h_t
                    if t == 3:
                        ddump("d_leaf3", leaf_t)
                    if t == 0:
                        ddump("d_h0", h_t)

                    feed(0, leaf_t)

            ctx = h_prev
            rt = root[0]
            ddump("d_ctx", ctx)
            ddump("d_root", rt)

            # ---------- tail: cmp, E-max, output head ----------
            with tc.tile_pool(name="tail", bufs=1) as tp:
                cpd = tp.tile([P, 2 * HT, NB], WD, name="cpd", tag="cpd")
                for ht in range(HT):
                    nc.vector.reduce_max(
                        out=cpd[:, HT + ht, :],
                        in_=ctx[:, ht, :].rearrange("p (b e) -> p b e", e=E),
                        axis=mybir.AxisListType.X,
                    )

                cmp_sb = tp.tile([P, HT, N], WD, name="cmp", tag="cmp")
                accc = [ps.tile([P, 512], F32, name="mm", tag="mm", bufs=8)[:, :N]
                        for _ in range(HT)]
                for mt in range(HT):
                    for kt in range(2 * HT):
                        src = rt if kt < HT else ctx
                        _mm(nc, accc[mt],
                            W["w_cmp"][:, kt, mt * P:(mt + 1) * P],
                            src[:, kt % HT, :],
                            start=(kt == 0), stop=(kt == 2 * HT - 1))
                for mt in range(HT):
                    nc.scalar.activation(
                        out=cmp_sb[:, mt, :], in_=accc[mt],
                        func=AF.Relu, bias=W["b_cmp"][:, mt: mt + 1],
                    )

                ddump("d_cmp", cmp_sb)
                for ht in range(HT):
                    nc.vector.reduce_max(
                        out=cpd[:, ht, :],
                        in_=cmp_sb[:, ht, :].rearrange("p (b e) -> p b e", e=E),
                        axis=mybir.AxisListType.X,
                    )

                ddump("d_cpd", cpd)
                o1_sb = tp.tile([P, HT, NB], WD, name="o1", tag="o1")
                acco = [ps.tile([P, 512], F32, name="mm", tag="mm", bufs=8)[:, :NB]
                        for _ in range(HT)]
                # ctx-half k-tiles (ready early) first, cmp-half last, so the
                # o1 matmuls start before the cmp reduce_max ops finish
                korder = list(range(HT, 2 * HT)) + list(range(HT))
                for mt in range(HT):
                    for j, kt in enumerate(korder):
                        _mm(nc, acco[mt],
                            W["w_o1"][:, kt, mt * P:(mt + 1) * P],
                            cpd[:, kt, :], start=(j == 0),
                            stop=(j == 2 * HT - 1))
                for mt in range(HT):
                    nc.scalar.activation(
                        out=o1_sb[:, mt, :], in_=acco[mt],
                        func=AF.Relu, bias=W["b_o1"][:, mt: mt + 1],
                    )

                ddump("d_o1", o1_sb)
                acc2 = ps.tile([P, 512], F32, name="mm", tag="mm",
                               bufs=8)[0:1, :NB]
                for kt in range(HT):
                    _mm(nc, acc2, W["w_o2"][:, kt, :], o1_sb[:, kt, :],
                        start=(kt == 0), stop=(kt == HT - 1))
                ex = tp.tile([1, NB], F32, name="ex", tag="ex")
                nc.scalar.activation(out=ex, in_=acc2, func=AF.Exp,
                                     bias=W["b_o2"][0:1, 0:1])
                ex1 = tp.tile([1, NB], F32, name="ex1", tag="ex1")
                nc.vector.tensor_scalar(out=ex1, in0=ex, scalar1=1.0,
                                        scalar2=None, op0=ALU.add)
                sp = tp.tile([1, NB], F32, name="sp", tag="sp")
                nc.scalar.activation(out=sp, in_=ex1, func=AF.Ln)
                nc.sync.dma_start(out=val[None, :], in_=sp[0:1, :])


# ---------------------------------------------------------------------------
# host side
# ---------------------------------------------------------------------------

_NC_CACHE = {}


def _get_nc(NB=NB_FULL, zb=False):
    key = (NB, zb)
    if key not in _NC_CACHE:
        _NC_CACHE[key] = build_nc(NB, zb=zb)
    return _NC_CACHE[key]


def make_in_maps(inputs, NB=NB_FULL, ncores=NCORES, wdt="bf16"):
    bf16 = ml_dtypes.bfloat16 if wdt == "bf16" else np.float32
    x = np.asarray(inputs["x"], dtype=np.float32)
    leaf_ids = np.asarray(inputs["leaf_ids"])

    def wb(name):
        return np.ascontiguousarray(np.asarray(inputs[name], np.float32).astype(bf16))

    def bf(name):
        return np.ascontiguousarray(np.asarray(inputs[name], np.float32))

    shared = {
        "emb": wb("emb"), "w_hole": wb("W_hole"), "w_fn": wb("W_fn"),
        "w_ih": wb("W_ih"), "w_hh": wb("W_hh"), "w_cmp": wb("W_cmp"),
        "w_o1": wb("W_o1"), "w_o2": wb("W_o2"),
        "b_hole": bf("b_hole"), "b_fn": bf("b_fn"), "b_rnn": bf("b_rnn"),
        "b_cmp": bf("b_cmp"), "b_o1": bf("b_o1"), "b_o2": bf("b_o2"),
        "iota": np.ascontiguousarray(
            (np.arange(P)[:, None] + P * np.arange(VT)[None, :]).astype(np.float32)
        ),
    }
    in_maps = []
    for c in range(ncores):
        xs = x[c * NB:(c + 1) * NB].reshape(NB * E, H)
        ids = leaf_ids[c * NB:(c + 1) * NB].astype(np.float32).ravel()
        m = dict(shared)
        m["xT"] = np.ascontiguousarray(xs.T.astype(bf16))
        m["idsf"] = np.ascontiguousarray(ids)
        in_maps.append(m)
    return in_maps


def kernel(**inputs):
    from concourse.bass_utils import run_bass_kernel_spmd

    zb = all(
        not np.any(np.asarray(inputs[b]))
        for b in ("b_hole", "b_fn", "b_rnn", "b_cmp", "b_o1", "b_o2")
    )
    nc = _get_nc(NB_FULL, zb=zb)
    in_maps = make_in_maps(inputs)
    res = run_bass_kernel_spmd(nc, in_maps, list(range(NCORES))).results
    out = np.concatenate([np.asarray(res[c]["val"]) for c in range(NCORES)])
    return out.astype(np.float32)


# revision 20
# speedup vs baseline: 1.0975x; 1.0159x over previous
"""Trainium2 Bass kernel for nn_ListREPLValueHead (tree NN + RNN value head).

Strategy
--------
Data-parallel over the sketch batch B=512 across 8 NeuronCores (64 sketches,
N = 64*8 = 512 example-columns per core).  All weights replicated.

All activations live in *transposed* layout [H on partitions, batch on free]:
every GEMM is then `out' = W.T-matmul(act')` with the native weight matrix as
the stationary lhsT operand and activations as the moving rhs — no transposes
anywhere on device (x is pre-transposed on host; the embedding gather is done
as a one-hot matmul so the gather result is produced directly transposed).

The whole datapath is bf16 (PSUM accumulation fp32): same PE throughput as
fp32r (1 cycle/row) but half the DMA bytes / SBUF footprint, 2x DVE, and much
better behaved on HW inside PSUM accumulation groups.  End-to-end
quantization error ~9e-3 vs the 2e-2 gate.

DMA priority order matters: one serial DMA stream, so the prologue-critical
tensors (iota, ids, emb, w_hole, xT) are issued first and emb is split into 4
chunks so the one-hot GEMM starts after chunk 0 (~3us).  Bulk weights follow,
tail weights (w_cmp/w_o1) prefetch during the main loop.

Pipeline per core:
  1. one-hot(leaf_ids) [V, B*L]  (DVE is_equal vs iota)
  2. embG' = emb.T @ onehot   [H, B*L]   (PE, ACT evacuates)
  3. xw'  = W_hole.T @ x' + b_hole  [H, N]  (PE + ACT)
  4. t = 0..15:   leaf_t' = relu(xw' + embG'[:, :, t] bcast-e) (DVE add,
                         Pool relu)
                  h_t' = tanh(W_ih.T leaf_t' + W_hh.T h_{t-1}' + b)
                         (PE two-pass: all W_ih k-tiles for the 4 m-tiles
                         first, then W_hh — hides the tanh ACT latency of
                         step t under the W_ih matmuls of step t+1)
                  after odd t: fold tree nodes streaming (PE + ACT relu),
                         O(depth) memory
  5. cmp' = relu(W_cmp.T [root'; ctx'] + b_cmp)   (concat via K-slicing)
  6. compared' = max over E  -> [2H, 64]          (DVE strided reduce)
  7. val = softplus(W_o2.T relu(W_o1.T compared' + b_o1) + b_o2)
     as Ln(1+Exp(.)) on ACT.

build_nc(reps>1) repeats the body with weights hoisted out of the loop —
bench-only mode measuring the steady-state compute time via dispatch slope.
"""

import numpy as np
import ml_dtypes

import concourse.bass as bass
import concourse.mybir as mybir
import concourse.tile as tile
from concourse import bacc

F32 = mybir.dt.float32
BF16 = mybir.dt.bfloat16
AF = mybir.ActivationFunctionType
ALU = mybir.AluOpType

NCORES = 8
B, E, L, H, V, D = 512, 8, 16, 512, 512, 4
P = 128
HT = H // P          # 4 h-tiles
VT = V // P          # 4 v-tiles
NB_FULL = B // NCORES  # 64 sketches per core

_WTILE = lambda ap_, kt: ap_.rearrange("(kt p) m -> p kt m", p=P)  # noqa: E731


def _mm(nc, psum, lhsT, rhs, start, stop):
    nc.tensor.matmul(out=psum, lhsT=lhsT, rhs=rhs, start=start, stop=stop)


def build_nc(NB=NB_FULL, reps=1, dbg=False, wdt="bf16", zb=False):  # zb kept for compat, unused
    """Emit the per-core program. NB = sketches per core (64 full size).
    reps>1: benchmark mode — repeat the body, weights loaded once."""
    N = NB * E
    NL = NB * L

    WD = BF16 if wdt == "bf16" else mybir.dt.float32r
    nc = bacc.Bacc("TRN2", target_bir_lowering=False, debug=False)

    dram = {}
    dram["xT"] = nc.dram_tensor("xT", [H, N], WD, kind="ExternalInput").ap()
    dram["idsf"] = nc.dram_tensor("idsf", [NL], F32, kind="ExternalInput").ap()
    dram["iota"] = nc.dram_tensor("iota", [P, VT], F32, kind="ExternalInput").ap()
    dram["emb"] = nc.dram_tensor("emb", [V, H], WD, kind="ExternalInput").ap()
    dram["w_hole"] = nc.dram_tensor("w_hole", [H, H], WD, kind="ExternalInput").ap()
    dram["w_fn"] = nc.dram_tensor("w_fn", [D, 2 * H, H], WD, kind="ExternalInput").ap()
    dram["w_ih"] = nc.dram_tensor("w_ih", [H, H], WD, kind="ExternalInput").ap()
    dram["w_hh"] = nc.dram_tensor("w_hh", [H, H], WD, kind="ExternalInput").ap()
    dram["w_cmp"] = nc.dram_tensor("w_cmp", [2 * H, H], WD, kind="ExternalInput").ap()
    dram["w_o1"] = nc.dram_tensor("w_o1", [2 * H, H], WD, kind="ExternalInput").ap()
    dram["w_o2"] = nc.dram_tensor("w_o2", [H, 1], WD, kind="ExternalInput").ap()
    for bn, shp in [("b_hole", [H]), ("b_fn", [D, H]), ("b_rnn", [H]),
                    ("b_cmp", [H]), ("b_o1", [H]), ("b_o2", [1])]:
        dram[bn] = nc.dram_tensor(bn, shp, F32, kind="ExternalInput").ap()
    val = nc.dram_tensor("val", [NB], F32, kind="ExternalOutput").ap()
    dbg_outs = {}
    if dbg:
        for nm, shape in [
            ("d_xw", [P, HT, N]), ("d_embG", [P, HT, NL]),
            ("d_leaf3", [P, HT, N]), ("d_h0", [P, HT, N]),
            ("d_ctx", [P, HT, N]), ("d_root", [P, HT, N]),
            ("d_cmp", [P, HT, N]), ("d_cpd", [P, 2 * HT, NB]),
            ("d_o1", [P, HT, NB]),
        ]:
            dbg_outs[nm] = nc.dram_tensor(nm, shape, WD, kind="ExternalOutput").ap()

    with tile.TileContext(nc) as tc:
        with (
            tc.tile_pool(name="persist", bufs=1) as pp,
            tc.tile_pool(name="psum", bufs=8, space="PSUM") as ps,
        ):
            if reps == 1:
                _emit(nc, tc, NB, WD, dbg_outs, pp, ps, dram, val, W=None, zb=zb)
            else:
                W = _load_weights(nc, pp, WD, dram, hoist_inputs=True)
                for _ in range(reps):
                    _emit(nc, tc, NB, WD, {}, pp, ps, dram, val, W=W, zb=zb)

    nc.compile()
    return nc


def _load_bias(nc, pool, ap_, tag):
    t = pool.tile([P, HT], F32, tag=tag)
    nc.sync.dma_start(out=t, in_=ap_.rearrange("(t p) -> p t", p=P))
    return t


def _load_weights(nc, pp, WD, dram, hoist_inputs=False):
    """Load everything weight-like into the persistent pool (bench mode:
    emb/w_hole also persist so the rep body only re-loads ids/xT)."""
    W = {}
    W["iota"] = pp.tile([P, VT], F32, name="iota", tag="iota")
    nc.sync.dma_start(out=W["iota"], in_=dram["iota"])
    for bn in ("b_hole", "b_rnn", "b_cmp", "b_o1"):
        W[bn] = _load_bias(nc, pp, dram[bn], bn)
    W["b_fn"] = pp.tile([P, D, HT], F32, name="b_fn", tag="b_fn")
    nc.sync.dma_start(out=W["b_fn"],
                      in_=dram["b_fn"].rearrange("d (t p) -> p d t", p=P))
    W["b_o2"] = pp.tile([1, 1], F32, name="b_o2", tag="b_o2")
    nc.sync.dma_start(out=W["b_o2"], in_=dram["b_o2"][None, :])
    if hoist_inputs:
        W["emb"] = pp.tile([P, VT, H], WD, name="emb", tag="emb")
        for vt in range(VT):
            nc.sync.dma_start(out=W["emb"][:, vt, :],
                              in_=dram["emb"][vt * P:(vt + 1) * P, :])
        W["w_hole"] = pp.tile([P, HT, H], WD, name="w_hole", tag="w_hole")
        nc.sync.dma_start(out=W["w_hole"], in_=_WTILE(dram["w_hole"], HT))
    W["w_ih"] = pp.tile([P, HT, H], WD, name="w_ih", tag="w_ih")
    nc.sync.dma_start(out=W["w_ih"], in_=_WTILE(dram["w_ih"], HT))
    W["w_hh"] = pp.tile([P, HT, H], WD, name="w_hh", tag="w_hh")
    nc.sync.dma_start(out=W["w_hh"], in_=_WTILE(dram["w_hh"], HT))
    W["w_fn"] = [pp.tile([P, 2 * HT, H], WD, name=f"w_fn{d}", tag=f"w_fn{d}")
                 for d in range(D)]
    for d in range(D):
        nc.sync.dma_start(out=W["w_fn"][d], in_=_WTILE(dram["w_fn"][d], 2 * HT))
    W["w_cmp"] = pp.tile([P, 2 * HT, H], WD, name="w_cmp", tag="w_cmp")
    nc.sync.dma_start(out=W["w_cmp"], in_=_WTILE(dram["w_cmp"], 2 * HT))
    W["w_o1"] = pp.tile([P, 2 * HT, H], WD, name="w_o1", tag="w_o1")
    nc.sync.dma_start(out=W["w_o1"], in_=_WTILE(dram["w_o1"], 2 * HT))
    W["w_o2"] = pp.tile([P, HT, 1], WD, name="w_o2", tag="w_o2")
    nc.sync.dma_start(out=W["w_o2"],
                      in_=dram["w_o2"].rearrange("(t p) o -> p t o", p=P))
    return W


def _emit(nc, tc, NB, WD, dbg_outs, pp, ps, dram, val, W=None, zb=False):
    N = NB * E
    NL = NB * L

    def ddump(nm, t):
        if nm in dbg_outs:
            nc.sync.dma_start(out=dbg_outs[nm], in_=t)

    with (
        tc.tile_pool(name="pro", bufs=1) as pro,
        tc.tile_pool(name="ohp", bufs=2) as ohp,
    ):
        # ---------- per-call inputs + (reps==1) priority-ordered weights ----
        if W is None:
            # graded path: interleave weight DMAs in critical-path order
            hoisted = False
            W = {}
            W["iota"] = pp.tile([P, VT], F32, name="iota", tag="iota")
            nc.sync.dma_start(out=W["iota"], in_=dram["iota"])
            # xw operands first (xw GEMM runs before the one-hot GEMM);
            # xT chunked per k-tile so the first matmul starts ~2us in.
            # ids only feeds the DVE is_equal masks (~8us in), so it loads
            # after the xw operands.
            w_hole_sb = pro.tile([P, HT, H], WD, name="w_hole", tag="w_hole")
            xT_sb = pro.tile([P, HT, N], WD, name="xT", tag="xT")
            for kt in range(HT):
                # interleave the two per-k-tile chunk streams so the kt=0
                # matmul's operands both arrive first
                nc.sync.dma_start(out=w_hole_sb[:, kt, :],
                                  in_=_WTILE(dram["w_hole"], HT)[:, kt, :])
                nc.sync.dma_start(out=xT_sb[:, kt, :],
                                  in_=_WTILE(dram["xT"], HT)[:, kt, :])
            ids_sb = pro.tile([P, NL], F32, name="ids", tag="ids")
            nc.sync.dma_start(out=ids_sb, in_=dram["idsf"].partition_broadcast(P))
            emb_sb = pro.tile([P, VT, H], WD, name="emb", tag="emb")
            for vt in range(VT):
                nc.sync.dma_start(out=emb_sb[:, vt, :],
                                  in_=dram["emb"][vt * P:(vt + 1) * P, :])
            for bn in ("b_hole", "b_rnn", "b_cmp", "b_o1"):
                W[bn] = _load_bias(nc, pp, dram[bn], bn)
            W["b_fn"] = pp.tile([P, D, HT], F32, name="b_fn", tag="b_fn")
            nc.sync.dma_start(out=W["b_fn"],
                              in_=dram["b_fn"].rearrange("d (t p) -> p d t", p=P))
            W["b_o2"] = pp.tile([1, 1], F32, name="b_o2", tag="b_o2")
            nc.sync.dma_start(out=W["b_o2"], in_=dram["b_o2"][None, :])
            W["w_ih"] = pp.tile([P, HT, H], WD, name="w_ih", tag="w_ih")
            nc.sync.dma_start(out=W["w_ih"], in_=_WTILE(dram["w_ih"], HT))
            W["w_hh"] = pp.tile([P, HT, H], WD, name="w_hh", tag="w_hh")
            nc.sync.dma_start(out=W["w_hh"], in_=_WTILE(dram["w_hh"], HT))
            W["w_fn"] = [pp.tile([P, 2 * HT, H], WD, name=f"w_fn{d}",
                                 tag=f"w_fn{d}") for d in range(D)]
            for d in range(D):
                nc.sync.dma_start(out=W["w_fn"][d],
                                  in_=_WTILE(dram["w_fn"][d], 2 * HT))
            W["w_cmp"] = pp.tile([P, 2 * HT, H], WD, name="w_cmp", tag="w_cmp")
            nc.sync.dma_start(out=W["w_cmp"], in_=_WTILE(dram["w_cmp"], 2 * HT))
            W["w_o1"] = pp.tile([P, 2 * HT, H], WD, name="w_o1", tag="w_o1")
            nc.sync.dma_start(out=W["w_o1"], in_=_WTILE(dram["w_o1"], 2 * HT))
            W["w_o2"] = pp.tile([P, HT, 1], WD, name="w_o2", tag="w_o2")
            nc.sync.dma_start(out=W["w_o2"],
                              in_=dram["w_o2"].rearrange("(t p) o -> p t o", p=P))
        else:
            hoisted = True
            ids_sb = pro.tile([P, NL], F32, name="ids", tag="ids")
            nc.sync.dma_start(out=ids_sb, in_=dram["idsf"].partition_broadcast(P))
            emb_sb = W["emb"]
            w_hole_sb = W["w_hole"]
            xT_sb = pro.tile([P, HT, N], WD, name="xT", tag="xT")
            nc.sync.dma_start(out=xT_sb, in_=_WTILE(dram["xT"], HT))

        # persistent activations (per-rep lifetime)
        with tc.tile_pool(name="actp", bufs=1) as ap_pool:
            xw_sb = ap_pool.tile([P, HT, N], WD, name="xw", tag="xw")
            embG_sb = ap_pool.tile([P, HT, NL], WD, name="embG", tag="embG")

            # ---------- prologue: xw, then one-hot -> embG ----------
            # xw GEMM first (kt-outer: only needs xT chunk kt), so PE starts
            # ~2us in; the is_equal masks compute on DVE meanwhile.
            accx = [ps.tile([P, 512], F32, name="mm", tag="mm", bufs=8)[:, :N]
                    for _ in range(HT)]
            for kt in range(HT):
                for mt in range(HT):
                    _mm(nc, accx[mt], w_hole_sb[:, kt, mt * P:(mt + 1) * P],
                        xT_sb[:, kt, :], start=(kt == 0), stop=(kt == HT - 1))
            for mt in range(HT):
                nc.scalar.activation(
                    out=xw_sb[:, mt, :], in_=accx[mt], func=AF.Identity,
                    bias=W["b_hole"][:, mt: mt + 1],
                )

            # one-hot GEMM ht-outer: embG[ht] evacuates as soon as its 2
            # PSUM groups close, so the t=0 leaf chain (DVE) overlaps the
            # remaining one-hot work instead of waiting for all of it.
            nhalf = (NL + 511) // 512
            ohs = []
            for vt in range(VT):
                oh = ohp.tile([P, NL], WD, name="oh", tag="oh", bufs=VT)
                nc.vector.tensor_scalar(
                    out=oh, in0=ids_sb, scalar1=W["iota"][:, vt: vt + 1],
                    scalar2=None, op0=ALU.is_equal,
                )
                ohs.append(oh)
            for ht in range(HT):
                eg = [ps.tile([P, 512], F32, name="mm", tag="mm", bufs=8)[:, : min(512, NL - 512 * h)]
                      for h in range(nhalf)]
                for vt in range(VT):
                    for hf in range(nhalf):
                        _mm(
                            nc, eg[hf],
                            emb_sb[:, vt, ht * P:(ht + 1) * P],
                            ohs[vt][:, 512 * hf: 512 * hf + eg[hf].shape[-1]],
                            start=(vt == 0), stop=(vt == VT - 1),
                        )
                for hf in range(nhalf):
                    w = eg[hf].shape[-1]
                    nc.scalar.activation(
                        out=embG_sb[:, ht, 512 * hf: 512 * hf + w],
                        in_=eg[hf], func=AF.Copy,
                    )

            ddump("d_xw", xw_sb)
            ddump("d_embG", embG_sb)
            # ---------- main loop: leaf / RNN / streaming tree ----------
            h_prev = None
            pending = [None] * D
            root = [None]

            with (
                tc.tile_pool(name="leafp", bufs=3) as leafp,
                tc.tile_pool(name="lvl0", bufs=2) as lvl0p,
                tc.tile_pool(name="lvl1", bufs=2) as lvl1p,
                tc.tile_pool(name="lvl2", bufs=2) as lvl2p,
            ):
                lvlp = [lvl0p, lvl1p, lvl2p]

                def emit_fn_node(d, left, right):
                    if d == D - 1:
                        out_t = ap_pool.tile([P, HT, N], WD, name="root", tag="root")
                    else:
                        out_t = lvlp[d].tile([P, HT, N], WD, name=f"lvl{d}",
                                             tag=f"lvl{d}")
                    accs = [ps.tile([P, 512], F32, name="mm", tag="mm", bufs=8)[:, :N]
                            for _ in range(HT)]
                    for mt in range(HT):
                        for kt in range(2 * HT):
                            src = left if kt < HT else right
                            _mm(nc, accs[mt],
                                W["w_fn"][d][:, kt, mt * P:(mt + 1) * P],
                                src[:, kt % HT, :],
                                start=(kt == 0), stop=(kt == 2 * HT - 1))
                    for mt in range(HT):
                        nc.scalar.activation(
                            out=out_t[:, mt, :], in_=accs[mt],
                            func=AF.Relu, bias=W["b_fn"][:, d, mt: mt + 1],
                        )
                    return out_t

                def feed(d, node):
                    while True:
                        if pending[d] is None:
                            pending[d] = node
                            return
                        left = pending[d]
                        pending[d] = None
                        node = emit_fn_node(d, left, node)
                        if d == D - 1:
                            root[0] = node
                            return
                        d += 1

                for t in range(L):
                    # per-ht add+relu on DVE so each W_ih k-tile can start as
                    # soon as its ht slice is ready.  NEVER use gpsimd here:
                    # the GPSIMD engine has a huge real per-op cost (~24us/op
                    # measured on HW) that the cost model misses.
                    leaf_t = leafp.tile([P, HT, N], WD, name="leaf", tag="leaf")
                    for ht in range(HT):
                        egb = (
                            embG_sb[:, ht, :]
                            .rearrange("p (b l) -> p b l", l=L)[:, :, t]
                            .broadcast_to((P, NB, E))
                        )
                        nc.vector.tensor_tensor(
                            out=leaf_t[:, ht, :].rearrange("p (b e) -> p b e", e=E),
                            in0=xw_sb[:, ht, :].rearrange("p (b e) -> p b e", e=E),
                            in1=egb, op=ALU.add,
                        )
                        nc.vector.tensor_scalar(
                            out=leaf_t[:, ht, :], in0=leaf_t[:, ht, :],
                            scalar1=0.0, scalar2=None, op0=ALU.max,
                        )

                    # RNN step, two-pass (hide tanh under next W_ih matmuls)
                    h_t = ap_pool.tile([P, HT, N], WD, name="h", tag="h", bufs=2)
                    accr = [ps.tile([P, 512], F32, name="mm", tag="mm", bufs=8)[:, :N]
                            for _ in range(HT)]
                    only_ih = h_prev is None
                    for mt in range(HT):
                        for kt in range(HT):
                            _mm(nc, accr[mt],
                                W["w_ih"][:, kt, mt * P:(mt + 1) * P],
                                leaf_t[:, kt, :], start=(kt == 0),
                                stop=(only_ih and kt == HT - 1))
                    if not only_ih:
                        for mt in range(HT):
                            for kt in range(HT):
                                _mm(nc, accr[mt],
                                    W["w_hh"][:, kt, mt * P:(mt + 1) * P],
                                    h_prev[:, kt, :],
                                    start=False, stop=(kt == HT - 1))
                    for mt in range(HT):
                        nc.scalar.activation(
                            out=h_t[:, mt, :], in_=accr[mt],
                            func=AF.Tanh, bias=W["b_rnn"][:, mt: mt + 1],
                        )
                    h_prev = h_t
                    if t == 3:
                        ddump("d_leaf3", leaf_t)
                    if t == 0:
                        ddump("d_h0", h_t)

                    feed(0, leaf_t)

            ctx = h_prev
            rt = root[0]
            ddump("d_ctx", ctx)
            ddump("d_root", rt)

            # ---------- tail: cmp, E-max, output head ----------
            with tc.tile_pool(name="tail", bufs=1) as tp:
                cpd = tp.tile([P, 2 * HT, NB], WD, name="cpd", tag="cpd")
                for ht in range(HT):
                    nc.vector.reduce_max(
                        out=cpd[:, HT + ht, :],
                        in_=ctx[:, ht, :].rearrange("p (b e) -> p b e", e=E),
                        axis=mybir.AxisListType.X,
                    )

                cmp_sb = tp.tile([P, HT, N], WD, name="cmp", tag="cmp")
                accc = [ps.tile([P, 512], F32, name="mm", tag="mm", bufs=8)[:, :N]
                        for _ in range(HT)]
                for mt in range(HT):
                    for kt in range(2 * HT):
                        src = rt if kt < HT else ctx
                        _mm(nc, accc[mt],
                            W["w_cmp"][:, kt, mt * P:(mt + 1) * P],
                            src[:, kt % HT, :],
                            start=(kt == 0), stop=(kt == 2 * HT - 1))
                for mt in range(HT):
                    nc.scalar.activation(
                        out=cmp_sb[:, mt, :], in_=accc[mt],
                        func=AF.Relu, bias=W["b_cmp"][:, mt: mt + 1],
                    )

                ddump("d_cmp", cmp_sb)
                for ht in range(HT):
                    nc.vector.reduce_max(
                        out=cpd[:, ht, :],
                        in_=cmp_sb[:, ht, :].rearrange("p (b e) -> p b e", e=E),
                        axis=mybir.AxisListType.X,
                    )

                ddump("d_cpd", cpd)
                o1_sb = tp.tile([P, HT, NB], WD, name="o1", tag="o1")
                acco = [ps.tile([P, 512], F32, name="mm", tag="mm", bufs=8)[:, :NB]
                        for _ in range(HT)]
                # ctx-half k-tiles (ready early) first, cmp-half last, so the
                # o1 matmuls start before the cmp reduce_max ops finish
                korder = list(range(HT, 2 * HT)) + list(range(HT))
                for mt in range(HT):
                    for j, kt in enumerate(korder):
                        _mm(nc, acco[mt],
                            W["w_o1"][:, kt, mt * P:(mt + 1) * P],
                            cpd[:, kt, :], start=(j == 0),
                            stop=(j == 2 * HT - 1))
                for mt in range(HT):
                    nc.scalar.activation(
                        out=o1_sb[:, mt, :], in_=acco[mt],
                        func=AF.Relu, bias=W["b_o1"][:, mt: mt + 1],
                    )

                ddump("d_o1", o1_sb)
                acc2 = ps.tile([P, 512], F32, name="mm", tag="mm",
                               bufs=8)[0:1, :NB]
                for kt in range(HT):
                    _mm(nc, acc2, W["w_o2"][:, kt, :], o1_sb[:, kt, :],
                        start=(kt == 0), stop=(kt == HT - 1))
                # softplus = Ln(exp(z + b_o2) + 1): the +1 rides the Ln op's
                # bias, keeping the whole tail chain on ACT (no DVE hop)
                ex = tp.tile([1, NB], F32, name="ex", tag="ex")
                nc.scalar.activation(out=ex, in_=acc2, func=AF.Exp,
                                     bias=W["b_o2"][0:1, 0:1])
                sp = tp.tile([1, NB], F32, name="sp", tag="sp")
                nc.scalar.activation(out=sp, in_=ex, func=AF.Ln, bias=1.0)
                nc.sync.dma_start(out=val[None, :], in_=sp[0:1, :])


# ---------------------------------------------------------------------------
# host side
# ---------------------------------------------------------------------------

_NC_CACHE = {}


def _get_nc(NB=NB_FULL, zb=False):
    key = (NB, zb)
    if key not in _NC_CACHE:
        _NC_CACHE[key] = build_nc(NB, zb=zb)
    return _NC_CACHE[key]


def make_in_maps(inputs, NB=NB_FULL, ncores=NCORES, wdt="bf16"):
    bf16 = ml_dtypes.bfloat16 if wdt == "bf16" else np.float32
    x = np.asarray(inputs["x"], dtype=np.float32)
    leaf_ids = np.asarray(inputs["leaf_ids"])

    def wb(name):
        return np.ascontiguousarray(np.asarray(inputs[name], np.float32).astype(bf16))

    def bf(name):
        return np.ascontiguousarray(np.asarray(inputs[name], np.float32))

    shared = {
        "emb": wb("emb"), "w_hole": wb("W_hole"), "w_fn": wb("W_fn"),
        "w_ih": wb("W_ih"), "w_hh": wb("W_hh"), "w_cmp": wb("W_cmp"),
        "w_o1": wb("W_o1"), "w_o2": wb("W_o2"),
        "b_hole": bf("b_hole"), "b_fn": bf("b_fn"), "b_rnn": bf("b_rnn"),
        "b_cmp": bf("b_cmp"), "b_o1": bf("b_o1"), "b_o2": bf("b_o2"),
        "iota": np.ascontiguousarray(
            (np.arange(P)[:, None] + P * np.arange(VT)[None, :]).astype(np.float32)
        ),
    }
    in_maps = []
    for c in range(ncores):
        xs = x[c * NB:(c + 1) * NB].reshape(NB * E, H)
        ids = leaf_ids[c * NB:(c + 1) * NB].astype(np.float32).ravel()
        m = dict(shared)
        m["xT"] = np.ascontiguousarray(xs.T.astype(bf16))
        m["idsf"] = np.ascontiguousarray(ids)
        in_maps.append(m)
    return in_maps


def kernel(**inputs):
    from concourse.bass_utils import run_bass_kernel_spmd

    zb = all(
        not np.any(np.asarray(inputs[b]))
        for b in ("b_hole", "b_fn", "b_rnn", "b_cmp", "b_o1", "b_o2")
    )
    nc = _get_nc(NB_FULL, zb=zb)
    in_maps = make_in_maps(inputs)
    res = run_bass_kernel_spmd(nc, in_maps, list(range(NCORES))).results
    out = np.concatenate([np.asarray(res[c]["val"]) for c in range(NCORES)])
    return out.astype(np.float32)
